# revision 20
# baseline (speedup 1.0000x reference)
"""Trainium2 Bass kernel for nn_EntropyGeoRouter.

Sharding: 8 cores; core c handles batch b=c//2, sequence-half h=c%2
(512 query rows, full 1024 keys of that batch). 4 query tiles of 128 rows.
One SPMD module for all cores; per-core differences live in input data
(host passes only index-derived tensors; all float math runs on device).
"""
import math
import os
import numpy as np
from contextlib import ExitStack

import concourse.bass as bass
import concourse.bacc as bacc
import concourse.tile as tile
from concourse import mybir, masks
from concourse._compat import with_exitstack
from concourse.bass_utils import run_bass_kernel_spmd

F32 = mybir.dt.float32
F32R = mybir.dt.float32r
BF16 = mybir.dt.bfloat16
I16 = mybir.dt.int16
U16 = mybir.dt.uint16
I32 = mybir.dt.int32
ALU = mybir.AluOpType
ACTF = mybir.ActivationFunctionType
AXX = mybir.AxisListType.X

B, S, D, NS, KNB, VOCAB = 4, 1024, 256, 64, 8, 32000
BIGNEG = 30000.0      # same-mode additive gate
MASKNEG = -50000.0    # blocked additive (mask|eye), pre-scaled for z*5
Q = 512               # query rows per core
NQT = 4               # query tiles per core
P = 128
C = 64                # match-pair capacity per query tile
LOGV = math.log(VOCAB)
RSQRT_MAGIC = 0x5F3759DF
STAGE = int(os.environ.get('KSTAGE', '5'))


def _rsqrt(nc, eng, pool, x, ncols):
    """rsqrt of positive [128, ncols] f32 via bit trick + 2 Newton steps."""
    y = pool.tile([P, ncols], F32, tag="nt_y")
    t = pool.tile([P, ncols], F32, tag="nt_t")
    u = pool.tile([P, ncols], F32, tag="nt_u")
    yi = y[:, :].bitcast(I32)
    xi = x[:, :].bitcast(I32)
    eng.tensor_scalar(yi, xi, 1, None, op0=ALU.arith_shift_right)
    eng.tensor_scalar(yi, yi, -1, RSQRT_MAGIC, op0=ALU.mult, op1=ALU.add)
    for _ in range(2):
        eng.tensor_tensor(t[:, :], y[:, :], y[:, :], op=ALU.mult)
        eng.tensor_tensor(u[:, :], t[:, :], x[:, :], op=ALU.mult)
        eng.tensor_scalar(u[:, :], u[:, :], -0.5, 1.5, op0=ALU.mult, op1=ALU.add)
        eng.tensor_tensor(y[:, :], y[:, :], u[:, :], op=ALU.mult)
    return y


@with_exitstack
def build_kernel(ctx: ExitStack, tc: tile.TileContext, io: dict, repeat: int = 1):
    nc = tc.nc

    singles = ctx.enter_context(tc.tile_pool(name="singles", bufs=1))
    persist = ctx.enter_context(tc.tile_pool(name="persist", bufs=1))
    work = ctx.enter_context(tc.tile_pool(name="work", bufs=2))
    small = ctx.enter_context(tc.tile_pool(name="small", bufs=3))
    nwt = ctx.enter_context(tc.tile_pool(name="newton", bufs=2))
    pz = ctx.enter_context(tc.tile_pool(name="pz", bufs=2, space="PSUM"))
    pw = ctx.enter_context(tc.tile_pool(name="pw", bufs=1, space="PSUM"))
    psm = ctx.enter_context(tc.tile_pool(name="psm", bufs=2, space="PSUM"))

    # --- identities ---
    ident = singles.tile([P, P], F32)
    masks.make_identity(nc, ident[:])
    ident_r = singles.tile([P, P], F32R)
    nc.vector.tensor_copy(ident_r[:], ident[:])
    ident_bf = singles.tile([P, P], BF16)
    masks.make_identity(nc, ident_bf[:])

    # --- broadcast params / constants ---
    def bcast(name, cols, dtype=F32, parts=P):
        t = singles.tile([parts, cols], dtype, name="bc_" + name)
        nc.sync.dma_start(t[:], io[name][:].partition_broadcast(parts))
        return t

    w_e1 = bcast("W_e1c", 8)     # col h = 0.5*W_e1[h]
    b_e1 = bcast("b_e1c", 8)     # 0.5*b_e1[h]
    w_e2 = bcast("W_e2c", 8)
    b_e2 = bcast("b_e2c", 1)
    b_gate = bcast("b_gatec", 1)
    rk8 = bcast("rk8c", 8)       # [1, 1/2, ..., 1/8]
    iota16 = bcast("iota16c", S, F32, C)   # iota16[j, t] = t
    mcol = singles.tile([C, 1], F32)      # 0..63 column
    nc.sync.dma_start(mcol[:], io["mcolc"][:])
    mcol65 = singles.tile([65, 1], F32)   # 0..63, row64 = -1
    nc.sync.dma_start(mcol65[:], io["mcol65c"][:])
    ohq_scale = singles.tile([65, 1], F32)  # +BIGNEG rows, -BIGNEG row 64
    nc.sync.dma_start(ohq_scale[:], io["ohqscalec"][:])
    ones64 = singles.tile([C, 1], F32)
    nc.vector.memset(ones64[:], 1.0)
    ones_row_bf = singles.tile([1, P], BF16)
    nc.vector.memset(ones_row_bf[:], 1.0)

    # W_gate broadcast row [128, 256]
    wgate_bc = bcast("W_gate", D)

    # W_velT [64, 256] fp32
    w_velT = persist.tile([64, D], F32)
    for i in range(2):
        src = work.tile([P, 64], F32, tag="ld64")
        nc.sync.dma_start(src[:], io["W_vel"][i * P : (i + 1) * P, :])
        pt = psm.tile([P, D], F32, tag="ps")
        nc.tensor.matmul(pt[:64, :P], src[:], ident[:], is_transpose=True)
        nc.scalar.copy(w_velT[:, i * P : (i + 1) * P], pt[:64, :P])

    # W_probeT [256,256] f32r as two [128,256] tiles (w_probeT[j][dT, d'])
    w_probeT = [persist.tile([P, D], F32R, name=f"wpT{i}") for i in range(2)]
    for i in range(2):
        src = work.tile([P, D], F32R, tag="ldr")
        nc.sync.dma_start(src[:], io["W_probe"][i * P : (i + 1) * P, :].bitcast(F32R))
        for j in range(2):
            pt = psm.tile([P, D], F32, tag="ps")
            nc.tensor.matmul(pt[:, :P].bitcast(F32R), src[:, j * P : (j + 1) * P],
                             ident_r[:], is_transpose=True)
            nc.vector.tensor_copy(w_probeT[j][:, i * P : (i + 1) * P],
                                  pt[:, :P].bitcast(F32R))

    # messages f32r copy for agg rhs: [128, 8*256], t-slab major
    msgs_r = persist.tile([P, 8 * D], F32R)
    for t in range(8):
        nc.sync.dma_start(msgs_r[:, t * D : (t + 1) * D],
                          io["messages"][t * P : (t + 1) * P, :].bitcast(F32R))

    # messagesT [256, 1024] fp32 as two [128, 1024] tiles (d-slab major)
    msgsT = [persist.tile([P, S], F32, name=f"msgsT{i}") for i in range(2)]
    for t in range(8):
        src = work.tile([P, D], F32, tag="ldm")
        nc.sync.dma_start(src[:], io["messages"][t * P : (t + 1) * P, :])
        for j in range(2):
            pt = psm.tile([P, D], F32, tag="ps")
            nc.tensor.matmul(pt[:, :P], src[:, j * P : (j + 1) * P], ident[:],
                             is_transpose=True)
            nc.scalar.copy(msgsT[j][:, t * P : (t + 1) * P], pt[:, :P])

    # scnT [128(pad), 1024] fp32 (rows 64..127 zero) + key modes
    scnT = persist.tile([P, S], F32)
    nc.vector.memset(scnT[64:, :], 0.0)
    mode_cols = persist.tile([P, 8], F32)
    for t in range(8):
        src = work.tile([P, 64], F32, tag="ld64")
        nc.sync.dma_start(src[:], io["scn"][t * P : (t + 1) * P, :])
        pt = psm.tile([P, D], F32, tag="ps")
        nc.tensor.matmul(pt[:64, :P], src[:], ident[:], is_transpose=True)
        nc.scalar.copy(scnT[:64, t * P : (t + 1) * P], pt[:64, :P])
        m8 = small.tile([P, 8], F32, tag="m8k")
        nc.vector.max(m8[:], src[:])
        mi = small.tile([P, 8], U16, tag="mik")
        nc.vector.max_index(mi[:], m8[:], src[:])
        nc.vector.tensor_copy(mode_cols[:, t : t + 1], mi[:, 0:1])
    # key mode row [1, 1024] -> broadcast -> one-hot [65, 1024] bf16
    ptm = psm.tile([P, D], F32, tag="ps")
    nc.tensor.matmul(ptm[:8, :P], mode_cols[:], ident[:], is_transpose=True)
    mrow8 = singles.tile([8, P], F32)
    nc.scalar.copy(mrow8[:], ptm[:8, :P])
    mode_row = singles.tile([1, S], F32)
    nc.sync.dma_start(mode_row[:].rearrange('a (b c) -> a b c', b=8), mrow8[:])
    mode_row_bf = singles.tile([1, S], BF16)
    nc.vector.tensor_copy(mode_row_bf[:], mode_row[:])
    mode_bc_ps = pz.tile([C, S], F32, tag="z")
    for h in range(2):
        nc.tensor.matmul(mode_bc_ps[:, h * 512 : (h + 1) * 512],
                         ones_row_bf[0:1, :C],
                         mode_row_bf[0:1, h * 512 : (h + 1) * 512],
                         start=True, stop=True, skip_group_check=True)
    ohk = persist.tile([65, S], BF16)
    nc.vector.tensor_scalar(ohk[:64, :], mode_bc_ps[:], mcol[:], None,
                            op0=ALU.is_equal)
    nc.vector.memset(ohk[64:65, :], 1.0)

    # scnT f32r view for the es matmul rhs
    scnT_r = persist.tile([64, S], F32R)
    nc.vector.tensor_copy(scnT_r[:], scnT[:64, :])

    out_dram = io["out"]

    # ---------------- per query tile ----------------
    for qt in [q for _ in range(repeat) for q in range(NQT)]:
        qsl = slice(qt * P, (qt + 1) * P)

        # -- loads --
        hid = work.tile([P, D], F32, tag="hid")
        nc.sync.dma_start(hid[:], io["hidden_q"][qsl, :])
        gv = work.tile([P, 64], F32, tag="gv")
        nc.sync.dma_start(gv[:], io["gv_q"][qsl, :])
        scnq = work.tile([P, 64], F32, tag="scnq")
        nc.sync.dma_start(scnq[:], io["scn_q"][qsl, :])
        mask_bf = work.tile([P, S], BF16, tag="mask")
        nc.sync.dma_start(mask_bf[:], io["maskneg_q"][qsl, :])
        ent = small.tile([P, 1], F32, tag="ent")
        nc.sync.dma_start(ent[:], io["ent_q"][qsl, :])
        conf = small.tile([P, 1], F32, tag="conf")
        nc.sync.dma_start(conf[:], io["conf_q"][qsl, :])
        pkcol = small.tile([C, 1], F32, tag="pkcol")
        nc.sync.dma_start(pkcol[:], io["pk_col"][qt, :, :])
        pk_w = small.tile([P, C // 16], U16, tag="pkw")
        nc.sync.dma_start(pk_w[:], io["pk_wrap"][qt, :, :])
        pq_w = small.tile([P, C // 16], U16, tag="pqw")
        nc.sync.dma_start(pq_w[:], io["pq_wrap"][qt, :, :])
        w4i = small.tile([P, 16], I16, tag="w4i")
        nc.sync.dma_start(w4i[:], io["w4_idx"][qt, :, :])
        s_oh = small.tile([C, P], F32, tag="soh")
        nc.sync.dma_start(s_oh[:], io["s_oh"][qt, :, :])
        k_oh = small.tile([C, 8], F32, tag="koh")
        nc.sync.dma_start(k_oh[:], io["k_oh"][qt, :, :])
        cov = small.tile([P, 1], F32, tag="cov")
        nc.sync.dma_start(cov[:], io["coverage"][qt, :, :])

        # -- hiddenT (f32r) --
        hidT = work.tile([P, 2 * P], F32R, tag="hidT")
        for j in range(2):
            pt = psm.tile([P, D], F32, tag="ps")
            nc.tensor.matmul(pt[:, :P], hid[:, j * P : (j + 1) * P], ident[:],
                             is_transpose=True)
            nc.vector.tensor_copy(hidT[:, j * P : (j + 1) * P],
                                  pt[:, :P].bitcast(F32R))

        # -- gw = sigmoid(hidden . W_gate + b_gate) * conf --
        gscr = work.tile([P, D], F32, tag="gscr")
        gacc = small.tile([P, 1], F32, tag="gacc")
        nc.vector.scalar_tensor_tensor(gscr[:], hid[:], 1.0, wgate_bc[:],
                                       op0=ALU.mult, op1=ALU.mult,
                                       accum_out=gacc[:])
        gw = small.tile([P, 1], F32, tag="gw")
        nc.scalar.activation(gw[:], gacc[:], ACTF.Sigmoid, bias=b_gate[:])
        nc.vector.tensor_tensor(gw[:], gw[:], conf[:], op=ALU.mult)
        one_m_gw = small.tile([P, 1], F32, tag="omg")
        nc.vector.tensor_scalar(one_m_gw[:], gw[:], -1.0, 1.0, op0=ALU.mult,
                                op1=ALU.add)
        aw = small.tile([P, 1], F32, tag="aw")
        nc.vector.tensor_tensor(aw[:], one_m_gw[:], cov[:], op=ALU.mult)
        bw = small.tile([P, 1], F32, tag="bw")
        nc.vector.tensor_scalar(bw[:], cov[:], -1.0, 1.0, op0=ALU.mult, op1=ALU.add)
        nc.vector.tensor_tensor(bw[:], one_m_gw[:], bw[:], op=ALU.mult)

        # -- endpoint pre + squared norm --
        ep = work.tile([P, 64], F32, tag="ep")
        nc.vector.scalar_tensor_tensor(ep[:], gv[:], 0.4, scnq[:],
                                       op0=ALU.mult, op1=ALU.add)
        sq_scr = work.tile([P, D], F32, tag="sqscr")
        ssq2 = small.tile([P, 2], F32, tag="ssq2")
        nc.scalar.activation(sq_scr[:, :64], ep[:], ACTF.Square,
                             accum_out=ssq2[:, 0:1])

        # -- h_pre = hidden + 0.3 * (gv @ W_vel^T), fp32 --
        gvT = work.tile([64, P], F32, tag="gvT")
        ptg = psm.tile([P, D], F32, tag="ps")
        nc.tensor.matmul(ptg[:64, :P], gv[:], ident[:], is_transpose=True)
        nc.scalar.copy(gvT[:], ptg[:64, :P])
        pvel = psm.tile([P, D], F32, tag="ps")
        nc.tensor.matmul(pvel[:], gvT[:], w_velT[:], start=True, stop=True)
        hpre = work.tile([P, D], F32, tag="hpre")
        nc.vector.scalar_tensor_tensor(hpre[:], pvel[:], 0.3, hid[:],
                                       op0=ALU.mult, op1=ALU.add)
        nc.scalar.activation(sq_scr[:], hpre[:], ACTF.Square,
                             accum_out=ssq2[:, 1:2])

        rsq2 = _rsqrt(nc, nc.vector, nwt, ssq2, 2)
        esc = small.tile([P, 1], F32, tag="esc")
        nc.vector.tensor_scalar(esc[:], ent[:], 5.0 / LOGV, None, op0=ALU.mult)
        nc.vector.tensor_tensor(esc[:], esc[:], rsq2[:, 0:1], op=ALU.mult)
        nc.vector.tensor_scalar_mul(ep[:], ep[:], esc[:])
        hsc = small.tile([P, 1], F32, tag="hsc")
        nc.vector.tensor_scalar(hsc[:], rsq2[:, 1:2], 2.5, None, op0=ALU.mult)
        nc.vector.tensor_scalar_mul(hpre[:], hpre[:], hsc[:])

        # -- transposes: epT (f32r), haT (fp32), scn_qT raw + x5 --
        epT = work.tile([64, P], F32R, tag="epT")
        pte = psm.tile([P, D], F32, tag="ps")
        nc.tensor.matmul(pte[:64, :P], ep[:], ident[:], is_transpose=True)
        nc.vector.tensor_copy(epT[:], pte[:64, :P].bitcast(F32R))
        haT = work.tile([P, 2 * P], F32, tag="haT")
        for j in range(2):
            pt = psm.tile([P, D], F32, tag="ps")
            nc.tensor.matmul(pt[:, :P], hpre[:, j * P : (j + 1) * P], ident[:],
                             is_transpose=True)
            nc.scalar.copy(haT[:, j * P : (j + 1) * P], pt[:, :P])
        sqT = work.tile([P, P], F32, tag="sqT")   # raw scn_qT (rows 64+ zero)
        nc.vector.memset(sqT[64:, :], 0.0)
        pts = psm.tile([P, D], F32, tag="ps")
        nc.tensor.matmul(pts[:64, :P], scnq[:], ident[:], is_transpose=True)
        nc.scalar.copy(sqT[:64, :], pts[:64, :P])
        sq5 = work.tile([64, P], F32, tag="sq5")
        nc.vector.tensor_scalar(sq5[:], sqT[:64, :], 5.0, None, op0=ALU.mult)

        # -- query mode one-hot [65, 128] bf16, scaled --
        m8q = small.tile([P, 8], F32, tag="m8q")
        nc.vector.max(m8q[:], scnq[:])
        miq = small.tile([P, 8], U16, tag="miq")
        nc.vector.max_index(miq[:], m8q[:], scnq[:])
        mqf = small.tile([P, 1], F32, tag="mqf")
        nc.vector.tensor_copy(mqf[:], miq[:, 0:1])
        ptq = psm.tile([P, D], F32, tag="ps")
        nc.tensor.matmul(ptq[:1, :P], mqf[:], ident[:], is_transpose=True)
        mrow_q = small.tile([1, P], BF16, tag="mrowq")
        nc.scalar.copy(mrow_q[:], ptq[:1, :P])
        mbq_ps = psm.tile([65, P], F32, tag="ps")
        nc.tensor.matmul(mbq_ps[:], ones_row_bf[0:1, :65], mrow_q[0:1, :],
                         start=True, stop=True, skip_group_check=True)
        ohq = work.tile([65, P], BF16, tag="ohq")
        nc.vector.tensor_scalar(ohq[:], mbq_ps[:], mcol65[:], ohq_scale[:],
                                op0=ALU.is_equal, op1=ALU.mult)
        nc.vector.memset(ohq[64:65, :], -BIGNEG)

        # ---- z_g PSUM: es(f32r) + hm(fp32) + mask(bf16) ----
        zg = pz.tile([P, S], F32, tag="z")
        for h in range(2):
            hs = slice(h * 512, (h + 1) * 512)
            nc.tensor.matmul(zg[:, hs], epT[:], scnT_r[:, hs],
                             start=True, stop=False, skip_group_check=True)
        for j in range(2):
            for h in range(2):
                hs = slice(h * 512, (h + 1) * 512)
                nc.tensor.matmul(zg[:, hs], haT[:, j * P : (j + 1) * P],
                                 msgsT[j][:, hs], start=False, stop=False,
                                 skip_group_check=True)
        for h in range(2):
            hs = slice(h * 512, (h + 1) * 512)
            nc.tensor.matmul(zg[:, hs], ident_bf[:], mask_bf[:, hs],
                             start=False, stop=True, skip_group_check=True)

        # ---- z_l PSUM: 5*scn_sim(fp32) + BIGNEG*(same-1) + mask ----
        zl = pz.tile([P, S], F32, tag="z")
        for h in range(2):
            hs = slice(h * 512, (h + 1) * 512)
            nc.tensor.matmul(zl[:, hs], sq5[:], scnT[:64, hs],
                             start=True, stop=False, skip_group_check=True)
        for h in range(2):
            hs = slice(h * 512, (h + 1) * 512)
            nc.tensor.matmul(zl[:, hs], ohq[:], ohk[:, hs],
                             start=False, stop=False, skip_group_check=True)
        for h in range(2):
            hs = slice(h * 512, (h + 1) * 512)
            nc.tensor.matmul(zl[:, hs], ident_bf[:], mask_bf[:, hs],
                             start=False, stop=True, skip_group_check=True)

        # ---- tau via top-8 ----
        def tau_of(zpsum, tag):
            t8 = small.tile([P, 8], F32, tag=tag + "t8")
            nc.vector.max(t8[:], zpsum[:])
            c8 = small.tile([P, 8], F32, tag=tag + "c8")
            d8 = small.tile([P, 8], F32, tag=tag + "d8")
            nc.vector.tensor_copy(c8[:, 0:1], t8[:, 0:1])
            nc.vector.tensor_tensor(c8[:, 1:8], t8[:, 1:8], t8[:, 0:7], op=ALU.add)
            nc.vector.tensor_copy(d8[:, 0:2], c8[:, 0:2])
            nc.vector.tensor_tensor(d8[:, 2:8], c8[:, 2:8], c8[:, 0:6], op=ALU.add)
            nc.vector.tensor_copy(c8[:, 0:4], d8[:, 0:4])
            nc.vector.tensor_tensor(c8[:, 4:8], d8[:, 4:8], d8[:, 0:4], op=ALU.add)
            nc.vector.tensor_scalar(c8[:], c8[:], -1.0, None, op0=ALU.add)
            nc.vector.tensor_tensor(c8[:], c8[:], rk8[:], op=ALU.mult)
            tau = small.tile([P, 1], F32, tag=tag + "tau")
            nc.vector.tensor_reduce(tau[:], c8[:], axis=AXX, op=ALU.max)
            return tau

        tau_g = tau_of(zg, "g")
        tau_l = tau_of(zl, "l")
        if STAGE < 2:
            outt = work.tile([P, D], F32, tag="outt")
            nc.vector.tensor_scalar_mul(outt[:], hid[:], tau_g[:])
            nc.vector.tensor_scalar_mul(outt[:], outt[:], tau_l[:])
            nc.sync.dma_start(out_dram[qsl, :], outt[:])
            continue

        # ---- W1' = relu(gw*zg - gw*tau_g), W2' = relu(bw*zl - bw*tau_l) ----
        nbias_g = small.tile([P, 1], F32, tag="nbg")
        nc.vector.tensor_tensor(nbias_g[:], gw[:], tau_g[:], op=ALU.mult)
        nc.vector.tensor_scalar(nbias_g[:], nbias_g[:], -1.0, None, op0=ALU.mult)
        nbias_l = small.tile([P, 1], F32, tag="nbl")
        nc.vector.tensor_tensor(nbias_l[:], bw[:], tau_l[:], op=ALU.mult)
        nc.vector.tensor_scalar(nbias_l[:], nbias_l[:], -1.0, None, op0=ALU.mult)
        w1 = work.tile([P, S], F32R, tag="w1")
        nc.scalar.activation(w1[:], zg[:], ACTF.Relu, bias=nbias_g[:], scale=gw[:])
        w2 = work.tile([P, S], F32R, tag="w2")
        nc.scalar.activation(w2[:], zl[:], ACTF.Relu, bias=nbias_l[:], scale=bw[:])

        if STAGE < 3:
            outt = work.tile([P, D], F32, tag="outt")
            nc.scalar.activation(outt[:], w1[:].bitcast(F32)[:, 0:D], ACTF.Copy)
            nc.vector.tensor_tensor(outt[:], outt[:], w2[:].bitcast(F32)[:, 0:D], op=ALU.add)
            nc.sync.dma_start(out_dram[qsl, :], outt[:])
            continue
        # ---- static branch ----
        g1 = work.tile([P, C], F32, tag="g1")
        nc.gpsimd.indirect_copy(g1[:], scnT[:, :], pk_w[:], True)
        g2 = work.tile([P, C], F32, tag="g2")
        nc.gpsimd.indirect_copy(g2[:], sqT[:, :], pq_w[:], True)
        pp = work.tile([C, C], F32, tag="pp")
        nc.vector.tensor_tensor(pp[:64, :], g1[:64, :], g2[:64, :], op=ALU.mult)
        ptsc = psm.tile([P, D], F32, tag="ps")
        nc.tensor.matmul(ptsc[:C, 0:1], pp[:64, :], ones64[:],
                         start=True, stop=True)
        simc = small.tile([C, 1], F32, tag="simc")
        nc.scalar.copy(simc[:], ptsc[:C, 0:1])
        krhs = small.tile([C, 8], F32, tag="krhs")
        nc.vector.tensor_scalar_mul(krhs[:], k_oh[:], simc[:])
        pss = psm.tile([P, D], F32, tag="ps")
        nc.tensor.matmul(pss[:, 0:8], s_oh[:], krhs[:], start=True, stop=True)
        sim_s = small.tile([P, 8], F32, tag="sims")
        nc.scalar.copy(sim_s[:], pss[:, 0:8])

        # edge MLP: gelu exact via erf; x2 = 0.5*(sim*W_e1[h] + b_e1[h])
        x2 = work.tile([P, 64], F32, tag="x2")
        for h in range(8):
            nc.vector.tensor_scalar(
                x2[:, h * 8 : (h + 1) * 8], sim_s[:], w_e1[:, h : h + 1],
                b_e1[:, h : h + 1], op0=ALU.mult, op1=ALU.add)
        # tanh-gelu: gelu(x) ~= x2*(1+tanh(c*(2*x2)*(1+0.17886*x2^2))), x2=x/2
        xsq = work.tile([P, 64], F32, tag="xsq")
        nc.vector.tensor_tensor(xsq[:], x2[:], x2[:], op=ALU.mult)
        nc.vector.tensor_scalar(xsq[:], xsq[:], 0.35772, 2.0, op0=ALU.mult,
                                op1=ALU.add)
        nc.vector.tensor_tensor(xsq[:], xsq[:], x2[:], op=ALU.mult)
        erf = work.tile([P, 64], F32, tag="erf")
        nc.scalar.activation(erf[:], xsq[:], ACTF.Tanh, scale=0.7978845608028654)
        edge = small.tile([P, 8], F32, tag="edge")
        he_h = small.tile([P, 8], F32, tag="heh")
        for h in range(8):
            nc.vector.scalar_tensor_tensor(he_h[:], erf[:, h * 8 : (h + 1) * 8],
                                           1.0, x2[:, h * 8 : (h + 1) * 8],
                                           op0=ALU.add, op1=ALU.mult)
            if h == 0:
                nc.vector.tensor_scalar(edge[:], he_h[:], w_e2[:, 0:1], b_e2[:],
                                        op0=ALU.mult, op1=ALU.add)
            else:
                nc.vector.scalar_tensor_tensor(edge[:], he_h[:],
                                               w_e2[:, h : h + 1], edge[:],
                                               op0=ALU.mult, op1=ALU.add)
        # softmax over k via sigmoid-exp identity
        mx = small.tile([P, 1], F32, tag="mx")
        nc.vector.tensor_reduce(mx[:], edge[:], axis=AXX, op=ALU.max)
        sg = small.tile([P, 8], F32, tag="sg")
        nc.scalar.activation(sg[:], edge[:], ACTF.Sigmoid, bias=mx[:], scale=-1.0)
        ex = small.tile([P, 8], F32, tag="ex")
        nc.vector.reciprocal(ex[:], sg[:])
        nc.vector.tensor_scalar(ex[:], ex[:], -1.0, None, op0=ALU.add)
        den = small.tile([P, 1], F32, tag="den")
        nc.vector.tensor_reduce(den[:], ex[:], axis=AXX, op=ALU.add)
        rden = small.tile([P, 1], F32, tag="rden")
        nc.vector.reciprocal(rden[:], den[:])
        ews = small.tile([P, 8], F32, tag="ews")
        nc.vector.tensor_scalar_mul(ews[:], ex[:], rden[:])
        # scatter weights: w4d layout slot-major [si*8 + k]
        w4d = small.tile([P, 16], BF16, tag="w4d")
        ewsaw = small.tile([P, 8], F32, tag="ewsaw")
        nc.vector.tensor_scalar_mul(ewsaw[:], ews[:], aw[:])
        nc.vector.tensor_copy(w4d[:, 0:8], ewsaw[:])
        nc.vector.tensor_copy(w4d[:, 8:16], ewsaw[:])
        w4 = work.tile([P, C], BF16, tag="w4")
        nc.gpsimd.local_scatter(w4[:], w4d[:], w4i[:], channels=P,
                                num_elems=C, num_idxs=16)
        ptw = psm.tile([P, D], BF16, tag="ps")
        nc.tensor.matmul(ptw[:C, :P], w4[:], ident_bf[:], is_transpose=True)
        w4T = small.tile([C, P], BF16, tag="w4T")
        nc.vector.tensor_copy(w4T[:], ptw[:C, :P])
        ohp = work.tile([C, S], BF16, tag="ohp")
        nc.vector.tensor_scalar(ohp[:], iota16[:], pkcol[:], None, op0=ALU.is_equal)

        if STAGE < 4:
            outt = work.tile([P, D], F32, tag="outt")
            nc.vector.tensor_scalar_mul(outt[:], hid[:], aw[:])
            nc.vector.tensor_tensor(outt[:, 0:8], outt[:, 0:8], ews[:], op=ALU.add)
            nc.vector.tensor_tensor(outt[:, 0:64], outt[:, 0:64], w4[:].bitcast(F32)[:, 0:32].bitcast(BF16), op=ALU.add)
            nc.sync.dma_start(out_dram[qsl, :], outt[:])
            continue
        # ---- WcT accumulation: W1'^T + W2'^T + static oh-matmul ----
        wcT_ps = pw.tile([P, S], F32, tag="wcT")
        for i in range(8):
            sl = slice(i * P, (i + 1) * P)
            nc.tensor.matmul(wcT_ps[:, sl].bitcast(F32R), w1[:, sl], ident_r[:],
                             is_transpose=True, start=True, stop=False,
                             skip_group_check=True)
            nc.tensor.matmul(wcT_ps[:, sl].bitcast(F32R), w2[:, sl], ident_r[:],
                             is_transpose=True, start=False, stop=False,
                             skip_group_check=True)
            nc.tensor.matmul(wcT_ps[:, sl], ohp[:, sl], w4T[:],
                             start=False, stop=True, skip_group_check=True)
        wcT = work.tile([P, S], F32R, tag="wcTs")
        nc.scalar.copy(wcT[:], wcT_ps[:].bitcast(F32R))

        if STAGE < 5:
            outt = work.tile([P, D], F32, tag="outt")
            nc.scalar.activation(outt[:], wcT[:].bitcast(F32)[:, 0:D], ACTF.Copy)
            nc.sync.dma_start(out_dram[qsl, :], outt[:])
            continue
        # ---- agg (f32r) and probe (f32r) ----
        agg = psm.tile([P, D], F32, tag="ps")
        for i in range(8):
            nc.tensor.matmul(agg[:], wcT[:, i * P : (i + 1) * P],
                             msgs_r[:, i * D : (i + 1) * D],
                             start=(i == 0), stop=(i == 7))
        probe = psm.tile([P, D], F32, tag="ps")
        for j in range(2):
            nc.tensor.matmul(probe[:], hidT[:, j * P : (j + 1) * P],
                             w_probeT[j][:], start=(j == 0), stop=(j == 1))

        # ---- rel = sigmoid(dot * rsqrt(na2*np2)); out = agg * rel ----
        probe_sb = work.tile([P, D], F32, tag="prsb")
        nc.scalar.copy(probe_sb[:], probe[:])
        na2 = small.tile([P, 1], F32, tag="na2")
        nc.scalar.activation(sq_scr[:], agg[:], ACTF.Square, accum_out=na2[:])
        np2 = small.tile([P, 1], F32, tag="np2")
        nc.scalar.activation(sq_scr[:], probe_sb[:], ACTF.Square, accum_out=np2[:])
        dot = small.tile([P, 1], F32, tag="dot")
        dscr = work.tile([P, D], F32, tag="dscr")
        nc.vector.scalar_tensor_tensor(dscr[:], agg[:], 1.0, probe_sb[:],
                                       op0=ALU.mult, op1=ALU.mult,
                                       accum_out=dot[:])
        nn2 = small.tile([P, 1], F32, tag="nn2")
        nc.vector.tensor_tensor(nn2[:], na2[:], np2[:], op=ALU.mult)
        rsqn = _rsqrt(nc, nc.vector, nwt, nn2, 1)
        rel = small.tile([P, 1], F32, tag="rel")
        nc.vector.tensor_tensor(rel[:], dot[:], rsqn[:], op=ALU.mult)
        nc.scalar.activation(rel[:], rel[:], ACTF.Sigmoid)
        outt = work.tile([P, D], F32, tag="outt")
        nc.vector.tensor_scalar_mul(outt[:], agg[:], rel[:])
        nc.sync.dma_start(out_dram[qsl, :], outt[:])


# ------------------------------------------------------------------ host ----

_CACHE = {}


def _build_module(repeat: int = 1):
    key = f"nc{repeat}"
    if key in _CACHE:
        return _CACHE[key]
    nc = bacc.Bacc("TRN2", target_bir_lowering=False, debug=False, num_devices=8)

    def dt_(name, shape, dtype, kind="ExternalInput"):
        return nc.dram_tensor(name, shape, dtype, kind=kind).ap()

    io = {
        "messages": dt_("messages", [S, D], F32),
        "scn": dt_("scn", [S, 64], F32),
        "scn_q": dt_("scn_q", [Q, 64], F32),
        "hidden_q": dt_("hidden_q", [Q, D], F32),
        "gv_q": dt_("gv_q", [Q, 64], F32),
        "maskneg_q": dt_("maskneg_q", [Q, S], BF16),
        "ent_q": dt_("ent_q", [Q, 1], F32),
        "conf_q": dt_("conf_q", [Q, 1], F32),
        "W_vel": dt_("W_vel", [D, NS], F32),
        "W_probe": dt_("W_probe", [D, D], F32),
        "W_gate": dt_("W_gate", [1, D], F32),
        "W_e1c": dt_("W_e1c", [1, 8], F32),
        "b_e1c": dt_("b_e1c", [1, 8], F32),
        "W_e2c": dt_("W_e2c", [1, 8], F32),
        "b_e2c": dt_("b_e2c", [1, 1], F32),
        "b_gatec": dt_("b_gatec", [1, 1], F32),
        "rk8c": dt_("rk8c", [1, 8], F32),
        "iota16c": dt_("iota16c", [1, S], F32),
        "mcolc": dt_("mcolc", [C, 1], F32),
        "mcol65c": dt_("mcol65c", [65, 1], F32),
        "ohqscalec": dt_("ohqscalec", [65, 1], F32),
        "pk_col": dt_("pk_col", [NQT, C, 1], F32),
        "pk_wrap": dt_("pk_wrap", [NQT, 128, C // 16], U16),
        "pq_wrap": dt_("pq_wrap", [NQT, 128, C // 16], U16),
        "w4_idx": dt_("w4_idx", [NQT, 128, 16], I16),
        "s_oh": dt_("s_oh", [NQT, C, 128], F32),
        "k_oh": dt_("k_oh", [NQT, C, 8], F32),
        "coverage": dt_("coverage", [NQT, 128, 1], F32),
        "out": dt_("out", [Q, D], F32, kind="ExternalOutput"),
    }
    with tile.TileContext(nc) as tc:
        build_kernel(tc, io, repeat)
    nc.compile()
    _CACHE[key] = nc
    return nc


def _host_prep(inputs):
    import ml_dtypes
    msgs = np.ascontiguousarray(inputs["messages"], dtype=np.float32)
    hid = np.ascontiguousarray(inputs["hidden"], dtype=np.float32)
    x_ids = np.asarray(inputs["x_ids"]).astype(np.int64)
    scn = np.ascontiguousarray(inputs["scn"], dtype=np.float32)
    mask = np.asarray(inputs["mask"]).astype(bool)
    static_nb = np.asarray(inputs["static_nb"]).astype(np.int64)
    gv = np.ascontiguousarray(inputs["geo_velocity"], dtype=np.float32)
    conf = np.asarray(inputs["ctx_conf"]).astype(np.float32).reshape(B, S, 1)
    ent = np.asarray(inputs["current_entropy"]).astype(np.float32)

    blocked = mask | np.eye(S, dtype=bool)
    maskneg = (MASKNEG * blocked.astype(np.float32)).astype(ml_dtypes.bfloat16)

    rk8 = (1.0 / np.arange(1, 9, dtype=np.float32)).reshape(1, 8)
    iota16 = np.arange(S, dtype=np.float32).reshape(1, S)
    mcol = np.arange(C, dtype=np.float32).reshape(C, 1)
    mcol65 = np.full((65, 1), -1.0, np.float32)
    mcol65[:64, 0] = np.arange(64)
    ohqsc = np.full((65, 1), BIGNEG, np.float32)
    ohqsc[64, 0] = -BIGNEG

    shared = {
        "W_vel": np.ascontiguousarray(inputs["W_vel"], dtype=np.float32),
        "W_probe": np.ascontiguousarray(inputs["W_probe"], dtype=np.float32),
        "W_gate": np.ascontiguousarray(inputs["W_gate"], dtype=np.float32).reshape(1, D),
        "W_e1c": (0.5 * np.asarray(inputs["W_e1"], np.float32)).reshape(1, 8),
        "b_e1c": (0.5 * np.asarray(inputs["b_e1"], np.float32)).reshape(1, 8),
        "W_e2c": np.asarray(inputs["W_e2"], np.float32).reshape(1, 8),
        "b_e2c": np.asarray(inputs["b_e2"], np.float32).reshape(1, 1),
        "b_gatec": np.asarray(inputs["b_gate"], np.float32).reshape(1, 1),
        "rk8c": rk8, "iota16c": iota16, "mcolc": mcol, "mcol65c": mcol65,
        "ohqscalec": ohqsc,
    }

    in_maps = []
    for core in range(8):
        b, half = divmod(core, 2)
        base = half * Q
        pos = {}
        for t, v in enumerate(x_ids[b]):
            pos.setdefault(int(v), []).append(t)
        nbv = static_nb[x_ids[b, base : base + Q]]

        pk_col = np.zeros((NQT, C, 1), np.float32)
        pk_wrap = np.zeros((NQT, 128, C // 16), np.uint16)
        pq_wrap = np.zeros((NQT, 128, C // 16), np.uint16)
        w4_idx = np.full((NQT, 128, 16), -1, np.int16)
        s_oh = np.zeros((NQT, C, 128), np.float32)
        k_oh = np.zeros((NQT, C, 8), np.float32)
        coverage = np.zeros((NQT, 128, 1), np.float32)

        for qt in range(NQT):
            pairs = []
            cnt = np.zeros((128, 8), np.float32)
            for s_loc in range(128):
                s_glob = base + qt * 128 + s_loc
                for k in range(8):
                    v = int(nbv[qt * 128 + s_loc, k])
                    ms = [t for t in pos.get(v, []) if t <= s_glob]
                    cnt[s_loc, k] = len(ms)
                    assert len(ms) <= 2, f"cnt>2: core{core} qt{qt}"
                    for si, t in enumerate(ms):
                        pairs.append((s_loc, k, t, si))
            assert len(pairs) <= C, f"{len(pairs)} pairs > C: core{core} qt{qt}"
            flatk = np.zeros(C, np.uint16)
            flatq = np.zeros(C, np.uint16)
            for j, (s_loc, k, t, si) in enumerate(pairs):
                pk_col[qt, j, 0] = t
                s_oh[qt, j, s_loc] = 1.0
                k_oh[qt, j, k] = 1.0
                w4_idx[qt, s_loc, si * 8 + k] = j
                flatk[j] = t
                flatq[j] = s_loc
            for p in range(128):
                for sw in range(C // 16):
                    pk_wrap[qt, p, sw] = flatk[sw * 16 + (p % 16)]
                    pq_wrap[qt, p, sw] = flatq[sw * 16 + (p % 16)]
            coverage[qt, :, 0] = np.minimum(cnt, 1.0).mean(-1)
            # unmatched pairs -> pk_col must not accidentally one-hot-match:
            for j in range(len(pairs), C):
                pk_col[qt, j, 0] = -1

        in_maps.append({
            **shared,
            "messages": msgs[b],
            "scn": scn[b],
            "scn_q": scn[b, base : base + Q],
            "hidden_q": hid[b, base : base + Q],
            "gv_q": gv[b, base : base + Q],
            "maskneg_q": np.ascontiguousarray(maskneg[base : base + Q]),
            "ent_q": ent[b, base : base + Q].reshape(Q, 1),
            "conf_q": conf[b, base : base + Q].reshape(Q, 1),
            "pk_col": pk_col, "pk_wrap": pk_wrap, "pq_wrap": pq_wrap,
            "w4_idx": w4_idx, "s_oh": s_oh, "k_oh": k_oh, "coverage": coverage,
        })
    return in_maps


def run(inputs, trace=False, repeat=1):
    in_maps = _host_prep(inputs)
    nc = _build_module(repeat)
    br = run_bass_kernel_spmd(nc, in_maps, list(range(8)), trace=trace)
    out = np.zeros((B, S, D), np.float32)
    for core in range(8):
        b, half = divmod(core, 2)
        out[b, half * Q : (half + 1) * Q] = br.results[core]["out"]
    return out, br


def kernel(**inputs):
    out, _ = run(inputs)
    return out


# revision 22
# speedup vs baseline: 1.0972x; 1.0972x over previous
"""Trainium2 Bass kernel for nn_EntropyGeoRouter.

Sharding: 8 cores; core c handles batch b=c//2, sequence-half h=c%2
(512 query rows, full 1024 keys of that batch). 4 query tiles of 128 rows.
One SPMD module for all cores; per-core differences live in input data
(host passes only index-derived tensors; all float math runs on device).
"""
import math
import os
import numpy as np
from contextlib import ExitStack

import concourse.bass as bass
import concourse.bacc as bacc
import concourse.tile as tile
from concourse import mybir, masks
from concourse._compat import with_exitstack
from concourse.bass_utils import run_bass_kernel_spmd

F32 = mybir.dt.float32
F32R = mybir.dt.float32r
BF16 = mybir.dt.bfloat16
I16 = mybir.dt.int16
U16 = mybir.dt.uint16
I32 = mybir.dt.int32
ALU = mybir.AluOpType
ACTF = mybir.ActivationFunctionType
AXX = mybir.AxisListType.X

B, S, D, NS, KNB, VOCAB = 4, 1024, 256, 64, 8, 32000
BIGNEG = 30000.0      # same-mode additive gate
MASKNEG = -50000.0    # blocked additive (mask|eye), pre-scaled for z*5
Q = 512               # query rows per core
NQT = 4               # query tiles per core
P = 128
C = 64                # match-pair capacity per query tile
LOGV = math.log(VOCAB)
RSQRT_MAGIC = 0x5F3759DF
STAGE = int(os.environ.get('KSTAGE', '5'))


def _rsqrt(nc, eng, pool, x, ncols):
    """rsqrt of positive [128, ncols] f32 via bit trick + 2 Newton steps."""
    y = pool.tile([P, ncols], F32, tag="nt_y")
    t = pool.tile([P, ncols], F32, tag="nt_t")
    u = pool.tile([P, ncols], F32, tag="nt_u")
    yi = y[:, :].bitcast(I32)
    xi = x[:, :].bitcast(I32)
    eng.tensor_scalar(yi, xi, 1, None, op0=ALU.arith_shift_right)
    eng.tensor_scalar(yi, yi, -1, RSQRT_MAGIC, op0=ALU.mult, op1=ALU.add)
    for _ in range(2):
        eng.tensor_tensor(t[:, :], y[:, :], y[:, :], op=ALU.mult)
        eng.tensor_tensor(u[:, :], t[:, :], x[:, :], op=ALU.mult)
        eng.tensor_scalar(u[:, :], u[:, :], -0.5, 1.5, op0=ALU.mult, op1=ALU.add)
        eng.tensor_tensor(y[:, :], y[:, :], u[:, :], op=ALU.mult)
    return y


@with_exitstack
def build_kernel(ctx: ExitStack, tc: tile.TileContext, io: dict, repeat: int = 1):
    nc = tc.nc

    singles = ctx.enter_context(tc.tile_pool(name="singles", bufs=1))
    persist = ctx.enter_context(tc.tile_pool(name="persist", bufs=1))
    work = ctx.enter_context(tc.tile_pool(name="work", bufs=4))
    small = ctx.enter_context(tc.tile_pool(name="small", bufs=4))
    nwt = ctx.enter_context(tc.tile_pool(name="newton", bufs=4))
    pz = ctx.enter_context(tc.tile_pool(name="pz", bufs=3, space="PSUM"))
    psm = ctx.enter_context(tc.tile_pool(name="psm", bufs=2, space="PSUM"))

    # --- identities ---
    ident = singles.tile([P, P], F32)
    masks.make_identity(nc, ident[:])
    ident_r = singles.tile([P, P], F32R)
    nc.vector.tensor_copy(ident_r[:], ident[:])
    ident_bf = singles.tile([P, P], BF16)
    masks.make_identity(nc, ident_bf[:])

    # --- broadcast params / constants ---
    def bcast(name, cols, dtype=F32, parts=P):
        t = singles.tile([parts, cols], dtype, name="bc_" + name)
        nc.sync.dma_start(t[:], io[name][:].partition_broadcast(parts))
        return t

    w_e1 = bcast("W_e1c", 8)     # col h = 0.5*W_e1[h]
    b_e1 = bcast("b_e1c", 8)     # 0.5*b_e1[h]
    w_e2 = bcast("W_e2c", 8)
    b_e2 = bcast("b_e2c", 1)
    b_gate = bcast("b_gatec", 1)
    rk8 = bcast("rk8c", 8)       # [1, 1/2, ..., 1/8]
    iota16 = bcast("iota16c", S, F32, C)   # iota16[j, t] = t
    mcol = singles.tile([C, 1], F32)      # 0..63 column
    nc.sync.dma_start(mcol[:], io["mcolc"][:])
    mcol65 = singles.tile([65, 1], F32)   # 0..63, row64 = -1
    nc.sync.dma_start(mcol65[:], io["mcol65c"][:])
    ohq_scale = singles.tile([65, 1], F32)  # +BIGNEG rows, -BIGNEG row 64
    nc.sync.dma_start(ohq_scale[:], io["ohqscalec"][:])
    ones64 = singles.tile([C, 1], F32)
    nc.vector.memset(ones64[:], 1.0)
    ones_row_bf = singles.tile([1, P], BF16)
    nc.vector.memset(ones_row_bf[:], 1.0)

    # W_gate broadcast row [128, 256]
    wgate_bc = bcast("W_gate", D)

    # W_velT [64, 256] fp32
    w_velT = persist.tile([64, D], F32)
    for i in range(2):
        src = work.tile([P, 64], F32, tag="ld64")
        nc.sync.dma_start(src[:], io["W_vel"][i * P : (i + 1) * P, :])
        pt = psm.tile([P, D], F32, tag="ps")
        nc.tensor.matmul(pt[:64, :P], src[:], ident[:], is_transpose=True)
        nc.scalar.copy(w_velT[:, i * P : (i + 1) * P], pt[:64, :P])

    # W_probeT [256,256] f32r as two [128,256] tiles (w_probeT[j][dT, d'])
    w_probeT = [persist.tile([P, D], F32R, name=f"wpT{i}") for i in range(2)]
    for i in range(2):
        src = work.tile([P, D], F32R, tag="ldr")
        nc.sync.dma_start(src[:], io["W_probe"][i * P : (i + 1) * P, :].bitcast(F32R))
        for j in range(2):
            pt = psm.tile([P, D], F32, tag="ps")
            nc.tensor.matmul(pt[:, :P].bitcast(F32R), src[:, j * P : (j + 1) * P],
                             ident_r[:], is_transpose=True)
            nc.vector.tensor_copy(w_probeT[j][:, i * P : (i + 1) * P],
                                  pt[:, :P].bitcast(F32R))

    # messages f32r copy for agg rhs: [128, 8*256], t-slab major
    msgs_r = persist.tile([P, 8 * D], F32R)
    for t in range(8):
        nc.sync.dma_start(msgs_r[:, t * D : (t + 1) * D],
                          io["messages"][t * P : (t + 1) * P, :].bitcast(F32R))

    # messagesT [256, 1024] fp32 as two [128, 1024] tiles (d-slab major)
    msgsT = [persist.tile([P, S], F32, name=f"msgsT{i}") for i in range(2)]
    for t in range(8):
        src = work.tile([P, D], F32, tag="ldm")
        nc.sync.dma_start(src[:], io["messages"][t * P : (t + 1) * P, :])
        for j in range(2):
            pt = psm.tile([P, D], F32, tag="ps")
            nc.tensor.matmul(pt[:, :P], src[:, j * P : (j + 1) * P], ident[:],
                             is_transpose=True)
            nc.scalar.copy(msgsT[j][:, t * P : (t + 1) * P], pt[:, :P])

    # scnT [128(pad), 1024] fp32 (rows 64..127 zero) + key modes
    scnT = persist.tile([P, S], F32)
    nc.vector.memset(scnT[64:, :], 0.0)
    mode_cols = persist.tile([P, 8], F32)
    for t in range(8):
        src = work.tile([P, 64], F32, tag="ld64")
        nc.sync.dma_start(src[:], io["scn"][t * P : (t + 1) * P, :])
        pt = psm.tile([P, D], F32, tag="ps")
        nc.tensor.matmul(pt[:64, :P], src[:], ident[:], is_transpose=True)
        nc.scalar.copy(scnT[:64, t * P : (t + 1) * P], pt[:64, :P])
        m8 = small.tile([P, 8], F32, tag="m8k")
        nc.vector.max(m8[:], src[:])
        mi = small.tile([P, 8], U16, tag="mik")
        nc.vector.max_index(mi[:], m8[:], src[:])
        nc.vector.tensor_copy(mode_cols[:, t : t + 1], mi[:, 0:1])
    # key mode row [1, 1024] -> broadcast -> one-hot [65, 1024] bf16
    ptm = psm.tile([P, D], F32, tag="ps")
    nc.tensor.matmul(ptm[:8, :P], mode_cols[:], ident[:], is_transpose=True)
    mrow8 = singles.tile([8, P], F32)
    nc.scalar.copy(mrow8[:], ptm[:8, :P])
    mode_row = singles.tile([1, S], F32)
    nc.sync.dma_start(mode_row[:].rearrange('a (b c) -> a b c', b=8), mrow8[:])
    mode_row_bf = singles.tile([1, S], BF16)
    nc.vector.tensor_copy(mode_row_bf[:], mode_row[:])
    mode_bc_ps = pz.tile([C, S], F32, tag="z")
    for h in range(2):
        nc.tensor.matmul(mode_bc_ps[:, h * 512 : (h + 1) * 512],
                         ones_row_bf[0:1, :C],
                         mode_row_bf[0:1, h * 512 : (h + 1) * 512],
                         start=True, stop=True, skip_group_check=True)
    ohk = persist.tile([65, S], BF16)
    nc.vector.tensor_scalar(ohk[:64, :], mode_bc_ps[:], mcol[:], None,
                            op0=ALU.is_equal)
    nc.vector.memset(ohk[64:65, :], 1.0)

    # scnT f32r view for the es matmul rhs
    scnT_r = persist.tile([64, S], F32R)
    nc.vector.tensor_copy(scnT_r[:], scnT[:64, :])

    out_dram = io["out"]

    # ---------------- per query tile ----------------
    for qt in [q for _ in range(repeat) for q in range(NQT)]:
        qsl = slice(qt * P, (qt + 1) * P)

        # -- loads --
        hid = work.tile([P, D], F32, tag="hid")
        nc.sync.dma_start(hid[:], io["hidden_q"][qsl, :])
        gv = work.tile([P, 64], F32, tag="gv")
        nc.sync.dma_start(gv[:], io["gv_q"][qsl, :])
        scnq = work.tile([P, 64], F32, tag="scnq")
        nc.sync.dma_start(scnq[:], io["scn_q"][qsl, :])
        mask_bf = work.tile([P, S], BF16, tag="mask")
        nc.sync.dma_start(mask_bf[:], io["maskneg_q"][qsl, :])
        ent = small.tile([P, 1], F32, tag="ent")
        nc.sync.dma_start(ent[:], io["ent_q"][qsl, :])
        conf = small.tile([P, 1], F32, tag="conf")
        nc.sync.dma_start(conf[:], io["conf_q"][qsl, :])
        pkcol = small.tile([C, 1], F32, tag="pkcol")
        nc.sync.dma_start(pkcol[:], io["pk_col"][qt, :, :])
        pk_w = small.tile([P, C // 16], U16, tag="pkw")
        nc.sync.dma_start(pk_w[:], io["pk_wrap"][qt, :, :])
        pq_w = small.tile([P, C // 16], U16, tag="pqw")
        nc.sync.dma_start(pq_w[:], io["pq_wrap"][qt, :, :])
        w4i = small.tile([P, 16], I16, tag="w4i")
        nc.sync.dma_start(w4i[:], io["w4_idx"][qt, :, :])
        s_oh = small.tile([C, P], F32, tag="soh")
        nc.sync.dma_start(s_oh[:], io["s_oh"][qt, :, :])
        k_oh = small.tile([C, 8], F32, tag="koh")
        nc.sync.dma_start(k_oh[:], io["k_oh"][qt, :, :])
        cov = small.tile([P, 1], F32, tag="cov")
        nc.sync.dma_start(cov[:], io["coverage"][qt, :, :])

        # -- hiddenT (f32r) --
        hidT = work.tile([P, 2 * P], F32R, tag="hidT")
        for j in range(2):
            pt = psm.tile([P, D], F32, tag="ps")
            nc.tensor.matmul(pt[:, :P], hid[:, j * P : (j + 1) * P], ident[:],
                             is_transpose=True)
            nc.vector.tensor_copy(hidT[:, j * P : (j + 1) * P],
                                  pt[:, :P].bitcast(F32R))

        # -- gw = sigmoid(hidden . W_gate + b_gate) * conf --
        gscr = work.tile([P, D], F32, tag="gscr")
        gacc = small.tile([P, 1], F32, tag="gacc")
        nc.vector.scalar_tensor_tensor(gscr[:], hid[:], 1.0, wgate_bc[:],
                                       op0=ALU.mult, op1=ALU.mult,
                                       accum_out=gacc[:])
        gw = small.tile([P, 1], F32, tag="gw")
        nc.scalar.activation(gw[:], gacc[:], ACTF.Sigmoid, bias=b_gate[:])
        nc.vector.tensor_tensor(gw[:], gw[:], conf[:], op=ALU.mult)
        one_m_gw = small.tile([P, 1], F32, tag="omg")
        nc.vector.tensor_scalar(one_m_gw[:], gw[:], -1.0, 1.0, op0=ALU.mult,
                                op1=ALU.add)
        aw = small.tile([P, 1], F32, tag="aw")
        nc.vector.tensor_tensor(aw[:], one_m_gw[:], cov[:], op=ALU.mult)
        bw = small.tile([P, 1], F32, tag="bw")
        nc.vector.tensor_scalar(bw[:], cov[:], -1.0, 1.0, op0=ALU.mult, op1=ALU.add)
        nc.vector.tensor_tensor(bw[:], one_m_gw[:], bw[:], op=ALU.mult)

        # -- endpoint pre + squared norm --
        ep = work.tile([P, 64], F32, tag="ep")
        nc.vector.scalar_tensor_tensor(ep[:], gv[:], 0.4, scnq[:],
                                       op0=ALU.mult, op1=ALU.add)
        sq_scr = work.tile([P, D], F32, tag="sqscr")
        ssq2 = small.tile([P, 2], F32, tag="ssq2")
        nc.scalar.activation(sq_scr[:, :64], ep[:], ACTF.Square,
                             accum_out=ssq2[:, 0:1])

        # -- h_pre = hidden + 0.3 * (gv @ W_vel^T), fp32 --
        gvT = work.tile([64, P], F32, tag="gvT")
        ptg = psm.tile([P, D], F32, tag="ps")
        nc.tensor.matmul(ptg[:64, :P], gv[:], ident[:], is_transpose=True)
        nc.scalar.copy(gvT[:], ptg[:64, :P])
        pvel = psm.tile([P, D], F32, tag="ps")
        nc.tensor.matmul(pvel[:], gvT[:], w_velT[:], start=True, stop=True)
        hpre = work.tile([P, D], F32, tag="hpre")
        nc.vector.scalar_tensor_tensor(hpre[:], pvel[:], 0.3, hid[:],
                                       op0=ALU.mult, op1=ALU.add)
        nc.scalar.activation(sq_scr[:], hpre[:], ACTF.Square,
                             accum_out=ssq2[:, 1:2])

        rsq2 = _rsqrt(nc, nc.vector, nwt, ssq2, 2)
        esc = small.tile([P, 1], F32, tag="esc")
        nc.vector.tensor_scalar(esc[:], ent[:], 5.0 / LOGV, None, op0=ALU.mult)
        nc.vector.tensor_tensor(esc[:], esc[:], rsq2[:, 0:1], op=ALU.mult)
        nc.vector.tensor_scalar_mul(ep[:], ep[:], esc[:])
        hsc = small.tile([P, 1], F32, tag="hsc")
        nc.vector.tensor_scalar(hsc[:], rsq2[:, 1:2], 2.5, None, op0=ALU.mult)
        nc.vector.tensor_scalar_mul(hpre[:], hpre[:], hsc[:])

        # -- transposes: epT (f32r), haT (fp32), scn_qT raw + x5 --
        epT = work.tile([64, P], F32R, tag="epT")
        pte = psm.tile([P, D], F32, tag="ps")
        nc.tensor.matmul(pte[:64, :P], ep[:], ident[:], is_transpose=True)
        nc.vector.tensor_copy(epT[:], pte[:64, :P].bitcast(F32R))
        haT = work.tile([P, 2 * P], F32, tag="haT")
        for j in range(2):
            pt = psm.tile([P, D], F32, tag="ps")
            nc.tensor.matmul(pt[:, :P], hpre[:, j * P : (j + 1) * P], ident[:],
                             is_transpose=True)
            nc.scalar.copy(haT[:, j * P : (j + 1) * P], pt[:, :P])
        sqT = work.tile([P, P], F32, tag="sqT")   # raw scn_qT (rows 64+ zero)
        nc.vector.memset(sqT[64:, :], 0.0)
        pts = psm.tile([P, D], F32, tag="ps")
        nc.tensor.matmul(pts[:64, :P], scnq[:], ident[:], is_transpose=True)
        nc.scalar.copy(sqT[:64, :], pts[:64, :P])
        sq5 = work.tile([64, P], F32, tag="sq5")
        nc.vector.tensor_scalar(sq5[:], sqT[:64, :], 5.0, None, op0=ALU.mult)

        # -- query mode one-hot [65, 128] bf16, scaled --
        m8q = small.tile([P, 8], F32, tag="m8q")
        nc.vector.max(m8q[:], scnq[:])
        miq = small.tile([P, 8], U16, tag="miq")
        nc.vector.max_index(miq[:], m8q[:], scnq[:])
        mqf = small.tile([P, 1], F32, tag="mqf")
        nc.vector.tensor_copy(mqf[:], miq[:, 0:1])
        ptq = psm.tile([P, D], F32, tag="ps")
        nc.tensor.matmul(ptq[:1, :P], mqf[:], ident[:], is_transpose=True)
        mrow_q = small.tile([1, P], BF16, tag="mrowq")
        nc.scalar.copy(mrow_q[:], ptq[:1, :P])
        mbq_ps = psm.tile([65, P], F32, tag="ps")
        nc.tensor.matmul(mbq_ps[:], ones_row_bf[0:1, :65], mrow_q[0:1, :],
                         start=True, stop=True, skip_group_check=True)
        ohq = work.tile([65, P], BF16, tag="ohq")
        nc.vector.tensor_scalar(ohq[:], mbq_ps[:], mcol65[:], ohq_scale[:],
                                op0=ALU.is_equal, op1=ALU.mult)
        nc.vector.memset(ohq[64:65, :], -BIGNEG)

        # ---- z_g PSUM: es(f32r) + hm(fp32) + mask(bf16) ----
        zg = pz.tile([P, S], F32, tag="z")
        for h in range(2):
            hs = slice(h * 512, (h + 1) * 512)
            nc.tensor.matmul(zg[:, hs], epT[:], scnT_r[:, hs],
                             start=True, stop=False, skip_group_check=True)
        for j in range(2):
            for h in range(2):
                hs = slice(h * 512, (h + 1) * 512)
                nc.tensor.matmul(zg[:, hs], haT[:, j * P : (j + 1) * P],
                                 msgsT[j][:, hs], start=False, stop=False,
                                 skip_group_check=True)
        for h in range(2):
            hs = slice(h * 512, (h + 1) * 512)
            nc.tensor.matmul(zg[:, hs], ident_bf[:], mask_bf[:, hs],
                             start=False, stop=True, skip_group_check=True)

        # ---- z_l PSUM: 5*scn_sim(fp32) + BIGNEG*(same-1) + mask ----
        zl = pz.tile([P, S], F32, tag="z")
        for h in range(2):
            hs = slice(h * 512, (h + 1) * 512)
            nc.tensor.matmul(zl[:, hs], sq5[:], scnT[:64, hs],
                             start=True, stop=False, skip_group_check=True)
        for h in range(2):
            hs = slice(h * 512, (h + 1) * 512)
            nc.tensor.matmul(zl[:, hs], ohq[:], ohk[:, hs],
                             start=False, stop=False, skip_group_check=True)
        for h in range(2):
            hs = slice(h * 512, (h + 1) * 512)
            nc.tensor.matmul(zl[:, hs], ident_bf[:], mask_bf[:, hs],
                             start=False, stop=True, skip_group_check=True)

        # ---- tau via top-8 ----
        def tau_of(zpsum, tag):
            t8 = small.tile([P, 8], F32, tag=tag + "t8")
            nc.vector.max(t8[:], zpsum[:])
            c8 = small.tile([P, 8], F32, tag=tag + "c8")
            d8 = small.tile([P, 8], F32, tag=tag + "d8")
            nc.vector.tensor_copy(c8[:, 0:1], t8[:, 0:1])
            nc.vector.tensor_tensor(c8[:, 1:8], t8[:, 1:8], t8[:, 0:7], op=ALU.add)
            nc.vector.tensor_copy(d8[:, 0:2], c8[:, 0:2])
            nc.vector.tensor_tensor(d8[:, 2:8], c8[:, 2:8], c8[:, 0:6], op=ALU.add)
            nc.vector.tensor_copy(c8[:, 0:4], d8[:, 0:4])
            nc.vector.tensor_tensor(c8[:, 4:8], d8[:, 4:8], d8[:, 0:4], op=ALU.add)
            nc.vector.tensor_scalar(c8[:], c8[:], -1.0, None, op0=ALU.add)
            nc.vector.tensor_tensor(c8[:], c8[:], rk8[:], op=ALU.mult)
            tau = small.tile([P, 1], F32, tag=tag + "tau")
            nc.vector.tensor_reduce(tau[:], c8[:], axis=AXX, op=ALU.max)
            return tau

        tau_g = tau_of(zg, "g")
        tau_l = tau_of(zl, "l")
        if STAGE < 2:
            outt = work.tile([P, D], F32, tag="outt")
            nc.vector.tensor_scalar_mul(outt[:], hid[:], tau_g[:])
            nc.vector.tensor_scalar_mul(outt[:], outt[:], tau_l[:])
            nc.sync.dma_start(out_dram[qsl, :], outt[:])
            continue

        # ---- W1' = relu(gw*zg - gw*tau_g), W2' = relu(bw*zl - bw*tau_l) ----
        nbias_g = small.tile([P, 1], F32, tag="nbg")
        nc.vector.tensor_tensor(nbias_g[:], gw[:], tau_g[:], op=ALU.mult)
        nc.vector.tensor_scalar(nbias_g[:], nbias_g[:], -1.0, None, op0=ALU.mult)
        nbias_l = small.tile([P, 1], F32, tag="nbl")
        nc.vector.tensor_tensor(nbias_l[:], bw[:], tau_l[:], op=ALU.mult)
        nc.vector.tensor_scalar(nbias_l[:], nbias_l[:], -1.0, None, op0=ALU.mult)
        w1 = work.tile([P, S], F32R, tag="w1")
        nc.scalar.activation(w1[:], zg[:], ACTF.Relu, bias=nbias_g[:], scale=gw[:])
        w2 = work.tile([P, S], F32R, tag="w2")
        nc.scalar.activation(w2[:], zl[:], ACTF.Relu, bias=nbias_l[:], scale=bw[:])

        if STAGE < 3:
            outt = work.tile([P, D], F32, tag="outt")
            nc.scalar.activation(outt[:], w1[:].bitcast(F32)[:, 0:D], ACTF.Copy)
            nc.vector.tensor_tensor(outt[:], outt[:], w2[:].bitcast(F32)[:, 0:D], op=ALU.add)
            nc.sync.dma_start(out_dram[qsl, :], outt[:])
            continue
        # ---- static branch ----
        g1 = work.tile([P, C], F32, tag="g1")
        nc.gpsimd.indirect_copy(g1[:], scnT[:, :], pk_w[:], True)
        g2 = work.tile([P, C], F32, tag="g2")
        nc.gpsimd.indirect_copy(g2[:], sqT[:, :], pq_w[:], True)
        pp = work.tile([C, C], F32, tag="pp")
        nc.vector.tensor_tensor(pp[:64, :], g1[:64, :], g2[:64, :], op=ALU.mult)
        ptsc = psm.tile([P, D], F32, tag="ps")
        nc.tensor.matmul(ptsc[:C, 0:1], pp[:64, :], ones64[:],
                         start=True, stop=True)
        simc = small.tile([C, 1], F32, tag="simc")
        nc.scalar.copy(simc[:], ptsc[:C, 0:1])
        krhs = small.tile([C, 8], F32, tag="krhs")
        nc.vector.tensor_scalar_mul(krhs[:], k_oh[:], simc[:])
        pss = psm.tile([P, D], F32, tag="ps")
        nc.tensor.matmul(pss[:, 0:8], s_oh[:], krhs[:], start=True, stop=True)
        sim_s = small.tile([P, 8], F32, tag="sims")
        nc.scalar.copy(sim_s[:], pss[:, 0:8])

        # edge MLP: gelu exact via erf; x2 = 0.5*(sim*W_e1[h] + b_e1[h])
        x2 = work.tile([P, 64], F32, tag="x2")
        for h in range(8):
            nc.vector.tensor_scalar(
                x2[:, h * 8 : (h + 1) * 8], sim_s[:], w_e1[:, h : h + 1],
                b_e1[:, h : h + 1], op0=ALU.mult, op1=ALU.add)
        # tanh-gelu: gelu(x) ~= x2*(1+tanh(c*(2*x2)*(1+0.17886*x2^2))), x2=x/2
        xsq = work.tile([P, 64], F32, tag="xsq")
        nc.vector.tensor_tensor(xsq[:], x2[:], x2[:], op=ALU.mult)
        nc.vector.tensor_scalar(xsq[:], xsq[:], 0.35772, 2.0, op0=ALU.mult,
                                op1=ALU.add)
        nc.vector.tensor_tensor(xsq[:], xsq[:], x2[:], op=ALU.mult)
        erf = work.tile([P, 64], F32, tag="erf")
        nc.scalar.activation(erf[:], xsq[:], ACTF.Tanh, scale=0.7978845608028654)
        edge = small.tile([P, 8], F32, tag="edge")
        he_h = small.tile([P, 8], F32, tag="heh")
        for h in range(8):
            nc.vector.scalar_tensor_tensor(he_h[:], erf[:, h * 8 : (h + 1) * 8],
                                           1.0, x2[:, h * 8 : (h + 1) * 8],
                                           op0=ALU.add, op1=ALU.mult)
            if h == 0:
                nc.vector.tensor_scalar(edge[:], he_h[:], w_e2[:, 0:1], b_e2[:],
                                        op0=ALU.mult, op1=ALU.add)
            else:
                nc.vector.scalar_tensor_tensor(edge[:], he_h[:],
                                               w_e2[:, h : h + 1], edge[:],
                                               op0=ALU.mult, op1=ALU.add)
        # softmax over k via sigmoid-exp identity
        mx = small.tile([P, 1], F32, tag="mx")
        nc.vector.tensor_reduce(mx[:], edge[:], axis=AXX, op=ALU.max)
        sg = small.tile([P, 8], F32, tag="sg")
        nc.scalar.activation(sg[:], edge[:], ACTF.Sigmoid, bias=mx[:], scale=-1.0)
        ex = small.tile([P, 8], F32, tag="ex")
        nc.vector.reciprocal(ex[:], sg[:])
        nc.vector.tensor_scalar(ex[:], ex[:], -1.0, None, op0=ALU.add)
        den = small.tile([P, 1], F32, tag="den")
        nc.vector.tensor_reduce(den[:], ex[:], axis=AXX, op=ALU.add)
        rden = small.tile([P, 1], F32, tag="rden")
        nc.vector.reciprocal(rden[:], den[:])
        ews = small.tile([P, 8], F32, tag="ews")
        nc.vector.tensor_scalar_mul(ews[:], ex[:], rden[:])
        # scatter weights: w4d layout slot-major [si*8 + k]
        w4d = small.tile([P, 16], BF16, tag="w4d")
        ewsaw = small.tile([P, 8], F32, tag="ewsaw")
        nc.vector.tensor_scalar_mul(ewsaw[:], ews[:], aw[:])
        nc.vector.tensor_copy(w4d[:, 0:8], ewsaw[:])
        nc.vector.tensor_copy(w4d[:, 8:16], ewsaw[:])
        w4 = work.tile([P, C], BF16, tag="w4")
        nc.gpsimd.local_scatter(w4[:], w4d[:], w4i[:], channels=P,
                                num_elems=C, num_idxs=16)
        ptw = psm.tile([P, D], BF16, tag="ps")
        nc.tensor.matmul(ptw[:C, :P], w4[:], ident_bf[:], is_transpose=True)
        w4T = small.tile([C, P], BF16, tag="w4T")
        nc.vector.tensor_copy(w4T[:], ptw[:C, :P])
        ohp = work.tile([C, S], BF16, tag="ohp")
        nc.vector.tensor_scalar(ohp[:], iota16[:], pkcol[:], None, op0=ALU.is_equal)

        if STAGE < 4:
            outt = work.tile([P, D], F32, tag="outt")
            nc.vector.tensor_scalar_mul(outt[:], hid[:], aw[:])
            nc.vector.tensor_tensor(outt[:, 0:8], outt[:, 0:8], ews[:], op=ALU.add)
            nc.vector.tensor_tensor(outt[:, 0:64], outt[:, 0:64], w4[:].bitcast(F32)[:, 0:32].bitcast(BF16), op=ALU.add)
            nc.sync.dma_start(out_dram[qsl, :], outt[:])
            continue
        # ---- WcT accumulation: W1'^T + W2'^T + static oh-matmul ----
        wcT_ps = pz.tile([P, S], F32, tag="z")
        for i in range(8):
            sl = slice(i * P, (i + 1) * P)
            nc.tensor.matmul(wcT_ps[:, sl].bitcast(F32R), w1[:, sl], ident_r[:],
                             is_transpose=True, start=True, stop=False,
                             skip_group_check=True)
            nc.tensor.matmul(wcT_ps[:, sl].bitcast(F32R), w2[:, sl], ident_r[:],
                             is_transpose=True, start=False, stop=False,
                             skip_group_check=True)
            nc.tensor.matmul(wcT_ps[:, sl], ohp[:, sl], w4T[:],
                             start=False, stop=True, skip_group_check=True)
        wcT = work.tile([P, S], F32R, tag="wcTs")
        nc.scalar.copy(wcT[:], wcT_ps[:].bitcast(F32R))

        if STAGE < 5:
            outt = work.tile([P, D], F32, tag="outt")
            nc.scalar.activation(outt[:], wcT[:].bitcast(F32)[:, 0:D], ACTF.Copy)
            nc.sync.dma_start(out_dram[qsl, :], outt[:])
            continue
        # ---- agg (f32r) and probe (f32r) ----
        agg = psm.tile([P, D], F32, tag="ps")
        for i in range(8):
            nc.tensor.matmul(agg[:], wcT[:, i * P : (i + 1) * P],
                             msgs_r[:, i * D : (i + 1) * D],
                             start=(i == 0), stop=(i == 7))
        probe = psm.tile([P, D], F32, tag="ps")
        for j in range(2):
            nc.tensor.matmul(probe[:], hidT[:, j * P : (j + 1) * P],
                             w_probeT[j][:], start=(j == 0), stop=(j == 1))

        # ---- rel = sigmoid(dot * rsqrt(na2*np2)); out = agg * rel ----
        probe_sb = work.tile([P, D], F32, tag="prsb")
        nc.scalar.copy(probe_sb[:], probe[:])
        na2 = small.tile([P, 1], F32, tag="na2")
        sq_scr2 = work.tile([P, D], F32, tag="sqscr2")
        nc.scalar.activation(sq_scr2[:], agg[:], ACTF.Square, accum_out=na2[:])
        np2 = small.tile([P, 1], F32, tag="np2")
        sq_scr3 = work.tile([P, D], F32, tag="sqscr3")
        nc.scalar.activation(sq_scr3[:], probe_sb[:], ACTF.Square, accum_out=np2[:])
        dot = small.tile([P, 1], F32, tag="dot")
        dscr = work.tile([P, D], F32, tag="dscr")
        nc.vector.scalar_tensor_tensor(dscr[:], agg[:], 1.0, probe_sb[:],
                                       op0=ALU.mult, op1=ALU.mult,
                                       accum_out=dot[:])
        nn2 = small.tile([P, 1], F32, tag="nn2")
        nc.vector.tensor_tensor(nn2[:], na2[:], np2[:], op=ALU.mult)
        rsqn = _rsqrt(nc, nc.vector, nwt, nn2, 1)
        rel = small.tile([P, 1], F32, tag="rel")
        nc.vector.tensor_tensor(rel[:], dot[:], rsqn[:], op=ALU.mult)
        nc.scalar.activation(rel[:], rel[:], ACTF.Sigmoid)
        outt = work.tile([P, D], F32, tag="outt")
        nc.vector.tensor_scalar_mul(outt[:], agg[:], rel[:])
        nc.sync.dma_start(out_dram[qsl, :], outt[:])


# ------------------------------------------------------------------ host ----

_CACHE = {}


def _build_module(repeat: int = 1):
    key = f"nc{repeat}"
    if key in _CACHE:
        return _CACHE[key]
    nc = bacc.Bacc("TRN2", target_bir_lowering=False, debug=False, num_devices=8)

    def dt_(name, shape, dtype, kind="ExternalInput"):
        return nc.dram_tensor(name, shape, dtype, kind=kind).ap()

    io = {
        "messages": dt_("messages", [S, D], F32),
        "scn": dt_("scn", [S, 64], F32),
        "scn_q": dt_("scn_q", [Q, 64], F32),
        "hidden_q": dt_("hidden_q", [Q, D], F32),
        "gv_q": dt_("gv_q", [Q, 64], F32),
        "maskneg_q": dt_("maskneg_q", [Q, S], BF16),
        "ent_q": dt_("ent_q", [Q, 1], F32),
        "conf_q": dt_("conf_q", [Q, 1], F32),
        "W_vel": dt_("W_vel", [D, NS], F32),
        "W_probe": dt_("W_probe", [D, D], F32),
        "W_gate": dt_("W_gate", [1, D], F32),
        "W_e1c": dt_("W_e1c", [1, 8], F32),
        "b_e1c": dt_("b_e1c", [1, 8], F32),
        "W_e2c": dt_("W_e2c", [1, 8], F32),
        "b_e2c": dt_("b_e2c", [1, 1], F32),
        "b_gatec": dt_("b_gatec", [1, 1], F32),
        "rk8c": dt_("rk8c", [1, 8], F32),
        "iota16c": dt_("iota16c", [1, S], F32),
        "mcolc": dt_("mcolc", [C, 1], F32),
        "mcol65c": dt_("mcol65c", [65, 1], F32),
        "ohqscalec": dt_("ohqscalec", [65, 1], F32),
        "pk_col": dt_("pk_col", [NQT, C, 1], F32),
        "pk_wrap": dt_("pk_wrap", [NQT, 128, C // 16], U16),
        "pq_wrap": dt_("pq_wrap", [NQT, 128, C // 16], U16),
        "w4_idx": dt_("w4_idx", [NQT, 128, 16], I16),
        "s_oh": dt_("s_oh", [NQT, C, 128], F32),
        "k_oh": dt_("k_oh", [NQT, C, 8], F32),
        "coverage": dt_("coverage", [NQT, 128, 1], F32),
        "out": dt_("out", [Q, D], F32, kind="ExternalOutput"),
    }
    with tile.TileContext(nc) as tc:
        build_kernel(tc, io, repeat)
    nc.compile()
    _CACHE[key] = nc
    return nc


def _host_prep(inputs):
    import ml_dtypes
    msgs = np.ascontiguousarray(inputs["messages"], dtype=np.float32)
    hid = np.ascontiguousarray(inputs["hidden"], dtype=np.float32)
    x_ids = np.asarray(inputs["x_ids"]).astype(np.int64)
    scn = np.ascontiguousarray(inputs["scn"], dtype=np.float32)
    mask = np.asarray(inputs["mask"]).astype(bool)
    static_nb = np.asarray(inputs["static_nb"]).astype(np.int64)
    gv = np.ascontiguousarray(inputs["geo_velocity"], dtype=np.float32)
    conf = np.asarray(inputs["ctx_conf"]).astype(np.float32).reshape(B, S, 1)
    ent = np.asarray(inputs["current_entropy"]).astype(np.float32)

    blocked = mask | np.eye(S, dtype=bool)
    maskneg = (MASKNEG * blocked.astype(np.float32)).astype(ml_dtypes.bfloat16)

    rk8 = (1.0 / np.arange(1, 9, dtype=np.float32)).reshape(1, 8)
    iota16 = np.arange(S, dtype=np.float32).reshape(1, S)
    mcol = np.arange(C, dtype=np.float32).reshape(C, 1)
    mcol65 = np.full((65, 1), -1.0, np.float32)
    mcol65[:64, 0] = np.arange(64)
    ohqsc = np.full((65, 1), BIGNEG, np.float32)
    ohqsc[64, 0] = -BIGNEG

    shared = {
        "W_vel": np.ascontiguousarray(inputs["W_vel"], dtype=np.float32),
        "W_probe": np.ascontiguousarray(inputs["W_probe"], dtype=np.float32),
        "W_gate": np.ascontiguousarray(inputs["W_gate"], dtype=np.float32).reshape(1, D),
        "W_e1c": (0.5 * np.asarray(inputs["W_e1"], np.float32)).reshape(1, 8),
        "b_e1c": (0.5 * np.asarray(inputs["b_e1"], np.float32)).reshape(1, 8),
        "W_e2c": np.asarray(inputs["W_e2"], np.float32).reshape(1, 8),
        "b_e2c": np.asarray(inputs["b_e2"], np.float32).reshape(1, 1),
        "b_gatec": np.asarray(inputs["b_gate"], np.float32).reshape(1, 1),
        "rk8c": rk8, "iota16c": iota16, "mcolc": mcol, "mcol65c": mcol65,
        "ohqscalec": ohqsc,
    }

    in_maps = []
    for core in range(8):
        b, half = divmod(core, 2)
        base = half * Q
        pos = {}
        for t, v in enumerate(x_ids[b]):
            pos.setdefault(int(v), []).append(t)
        nbv = static_nb[x_ids[b, base : base + Q]]

        pk_col = np.zeros((NQT, C, 1), np.float32)
        pk_wrap = np.zeros((NQT, 128, C // 16), np.uint16)
        pq_wrap = np.zeros((NQT, 128, C // 16), np.uint16)
        w4_idx = np.full((NQT, 128, 16), -1, np.int16)
        s_oh = np.zeros((NQT, C, 128), np.float32)
        k_oh = np.zeros((NQT, C, 8), np.float32)
        coverage = np.zeros((NQT, 128, 1), np.float32)

        for qt in range(NQT):
            pairs = []
            cnt = np.zeros((128, 8), np.float32)
            for s_loc in range(128):
                s_glob = base + qt * 128 + s_loc
                for k in range(8):
                    v = int(nbv[qt * 128 + s_loc, k])
                    ms = [t for t in pos.get(v, []) if t <= s_glob]
                    cnt[s_loc, k] = len(ms)
                    assert len(ms) <= 2, f"cnt>2: core{core} qt{qt}"
                    for si, t in enumerate(ms):
                        pairs.append((s_loc, k, t, si))
            assert len(pairs) <= C, f"{len(pairs)} pairs > C: core{core} qt{qt}"
            flatk = np.zeros(C, np.uint16)
            flatq = np.zeros(C, np.uint16)
            for j, (s_loc, k, t, si) in enumerate(pairs):
                pk_col[qt, j, 0] = t
                s_oh[qt, j, s_loc] = 1.0
                k_oh[qt, j, k] = 1.0
                w4_idx[qt, s_loc, si * 8 + k] = j
                flatk[j] = t
                flatq[j] = s_loc
            for p in range(128):
                for sw in range(C // 16):
                    pk_wrap[qt, p, sw] = flatk[sw * 16 + (p % 16)]
                    pq_wrap[qt, p, sw] = flatq[sw * 16 + (p % 16)]
            coverage[qt, :, 0] = np.minimum(cnt, 1.0).mean(-1)
            # unmatched pairs -> pk_col must not accidentally one-hot-match:
            for j in range(len(pairs), C):
                pk_col[qt, j, 0] = -1

        in_maps.append({
            **shared,
            "messages": msgs[b],
            "scn": scn[b],
            "scn_q": scn[b, base : base + Q],
            "hidden_q": hid[b, base : base + Q],
            "gv_q": gv[b, base : base + Q],
            "maskneg_q": np.ascontiguousarray(maskneg[base : base + Q]),
            "ent_q": ent[b, base : base + Q].reshape(Q, 1),
            "conf_q": conf[b, base : base + Q].reshape(Q, 1),
            "pk_col": pk_col, "pk_wrap": pk_wrap, "pq_wrap": pq_wrap,
            "w4_idx": w4_idx, "s_oh": s_oh, "k_oh": k_oh, "coverage": coverage,
        })
    return in_maps


def run(inputs, trace=False, repeat=1):
    in_maps = _host_prep(inputs)
    nc = _build_module(repeat)
    br = run_bass_kernel_spmd(nc, in_maps, list(range(8)), trace=trace)
    out = np.zeros((B, S, D), np.float32)
    for core in range(8):
        b, half = divmod(core, 2)
        out[b, half * Q : (half + 1) * Q] = br.results[core]["out"]
    return out, br


def kernel(**inputs):
    out, _ = run(inputs)
    return out


# revision 26
# speedup vs baseline: 1.2281x; 1.1193x over previous
"""Trainium2 Bass kernel for nn_EntropyGeoRouter.

Sharding: 8 cores; core c handles batch b=c//2, sequence-half h=c%2
(512 query rows, full 1024 keys of that batch). 4 query tiles of 128 rows.
One SPMD module for all cores; per-core differences live in input data
(host passes only index-derived tensors; all float math runs on device).
"""
import math
import os
import numpy as np
from contextlib import ExitStack

import concourse.bass as bass
import concourse.bacc as bacc
import concourse.tile as tile
from concourse import mybir, masks
from concourse._compat import with_exitstack
from concourse.bass_utils import run_bass_kernel_spmd

F32 = mybir.dt.float32
F32R = mybir.dt.float32r
BF16 = mybir.dt.bfloat16
I16 = mybir.dt.int16
U16 = mybir.dt.uint16
I32 = mybir.dt.int32
ALU = mybir.AluOpType
ACTF = mybir.ActivationFunctionType
AXX = mybir.AxisListType.X

B, S, D, NS, KNB, VOCAB = 4, 1024, 256, 64, 8, 32000
BIGNEG = 30000.0      # same-mode additive gate
MASKNEG = -50000.0    # blocked additive (mask|eye), pre-scaled for z*5
Q = 512               # query rows per core
NQT = 4               # query tiles per core
P = 128
C = 64                # match-pair capacity per query tile
LOGV = math.log(VOCAB)
RSQRT_MAGIC = 0x5F3759DF
STAGE = int(os.environ.get('KSTAGE', '5'))


def _rsqrt(nc, eng, pool, x, ncols):
    """rsqrt of positive [128, ncols] f32 via bit trick + 2 Newton steps."""
    y = pool.tile([P, ncols], F32, tag="nt_y")
    t = pool.tile([P, ncols], F32, tag="nt_t")
    u = pool.tile([P, ncols], F32, tag="nt_u")
    yi = y[:, :].bitcast(I32)
    xi = x[:, :].bitcast(I32)
    eng.tensor_scalar(yi, xi, 1, None, op0=ALU.arith_shift_right)
    eng.tensor_scalar(yi, yi, -1, RSQRT_MAGIC, op0=ALU.mult, op1=ALU.add)
    for _ in range(2):
        eng.tensor_tensor(t[:, :], y[:, :], y[:, :], op=ALU.mult)
        eng.tensor_tensor(u[:, :], t[:, :], x[:, :], op=ALU.mult)
        eng.tensor_scalar(u[:, :], u[:, :], -0.5, 1.5, op0=ALU.mult, op1=ALU.add)
        eng.tensor_tensor(y[:, :], y[:, :], u[:, :], op=ALU.mult)
    return y


@with_exitstack
def build_kernel(ctx: ExitStack, tc: tile.TileContext, io: dict, repeat: int = 1):
    nc = tc.nc

    singles = ctx.enter_context(tc.tile_pool(name="singles", bufs=1))
    persist = ctx.enter_context(tc.tile_pool(name="persist", bufs=1))
    work = ctx.enter_context(tc.tile_pool(name="work", bufs=4))
    small = ctx.enter_context(tc.tile_pool(name="small", bufs=4))
    nwt = ctx.enter_context(tc.tile_pool(name="newton", bufs=4))
    pz = ctx.enter_context(tc.tile_pool(name="pz", bufs=2, space="PSUM"))
    psm = ctx.enter_context(tc.tile_pool(name="psm", bufs=4, space="PSUM"))

    # --- identities ---
    ident = singles.tile([P, P], F32)
    masks.make_identity(nc, ident[:])
    ident_r = singles.tile([P, P], F32R)
    nc.vector.tensor_copy(ident_r[:], ident[:])
    ident_bf = singles.tile([P, P], BF16)
    masks.make_identity(nc, ident_bf[:])

    # --- broadcast params / constants ---
    def bcast(name, cols, dtype=F32, parts=P):
        t = singles.tile([parts, cols], dtype, name="bc_" + name)
        nc.sync.dma_start(t[:], io[name][:].partition_broadcast(parts))
        return t

    w_e1 = bcast("W_e1c", 64)   # h-major, each 0.5*W_e1[h] x8
    b_e1 = bcast("b_e1c", 64)
    w_e2 = bcast("W_e2c", 64)
    b_e2 = bcast("b_e2c", 1)
    b_gate = bcast("b_gatec", 1)
    rk8 = bcast("rk8c", 8)       # [1, 1/2, ..., 1/8]
    iota16 = bcast("iota16c", S, F32, C)   # iota16[j, t] = t
    mcol = singles.tile([C, 1], F32)      # 0..63 column
    nc.sync.dma_start(mcol[:], io["mcolc"][:])
    mcol65 = singles.tile([65, 1], F32)   # 0..63, row64 = -1
    nc.sync.dma_start(mcol65[:], io["mcol65c"][:])
    ohq_scale = singles.tile([65, 1], F32)  # +BIGNEG rows, -BIGNEG row 64
    nc.sync.dma_start(ohq_scale[:], io["ohqscalec"][:])
    ones64 = singles.tile([C, 1], F32)
    nc.vector.memset(ones64[:], 1.0)
    ones_row_bf = singles.tile([1, P], BF16)
    nc.vector.memset(ones_row_bf[:], 1.0)

    # W_gate broadcast row [128, 256]
    wgate_bc = bcast("W_gate", D)

    # W_velT [64, 256] fp32
    w_velT = persist.tile([64, D], F32)
    for i in range(2):
        src = work.tile([P, 64], F32, tag="ld64")
        nc.sync.dma_start(src[:], io["W_vel"][i * P : (i + 1) * P, :])
        pt = psm.tile([P, D], F32, tag="ps")
        nc.tensor.matmul(pt[:64, :P], src[:], ident[:], is_transpose=True)
        nc.scalar.copy(w_velT[:, i * P : (i + 1) * P], pt[:64, :P])

    # W_probeT [256,256] f32r as two [128,256] tiles (w_probeT[j][dT, d'])
    w_probeT = [persist.tile([P, D], F32R, name=f"wpT{i}") for i in range(2)]
    for i in range(2):
        src = work.tile([P, D], F32R, tag="ldr")
        nc.sync.dma_start(src[:], io["W_probe"][i * P : (i + 1) * P, :].bitcast(F32R))
        for j in range(2):
            pt = psm.tile([P, D], F32, tag="ps")
            nc.tensor.matmul(pt[:, :P].bitcast(F32R), src[:, j * P : (j + 1) * P],
                             ident_r[:], is_transpose=True)
            nc.vector.tensor_copy(w_probeT[j][:, i * P : (i + 1) * P],
                                  pt[:, :P].bitcast(F32R))

    # messages f32r copy for agg rhs: [128, 8*256], t-slab major
    msgs_r = persist.tile([P, 8 * D], F32R)
    for t in range(8):
        nc.sync.dma_start(msgs_r[:, t * D : (t + 1) * D],
                          io["messages"][t * P : (t + 1) * P, :].bitcast(F32R))

    # messagesT [256, 1024] fp32 as two [128, 1024] tiles (d-slab major)
    msgsT = [persist.tile([P, S], F32, name=f"msgsT{i}") for i in range(2)]
    for t in range(8):
        src = work.tile([P, D], F32, tag="ldm")
        nc.sync.dma_start(src[:], io["messages"][t * P : (t + 1) * P, :])
        for j in range(2):
            pt = psm.tile([P, D], F32, tag="ps")
            nc.tensor.matmul(pt[:, :P], src[:, j * P : (j + 1) * P], ident[:],
                             is_transpose=True)
            nc.scalar.copy(msgsT[j][:, t * P : (t + 1) * P], pt[:, :P])

    # scnT [128(pad), 1024] fp32 (rows 64..127 zero) + key modes
    scnT = persist.tile([P, S], F32)
    nc.vector.memset(scnT[64:, :], 0.0)
    mode_cols = persist.tile([P, 8], F32)
    for t in range(8):
        src = work.tile([P, 64], F32, tag="ld64")
        nc.sync.dma_start(src[:], io["scn"][t * P : (t + 1) * P, :])
        pt = psm.tile([P, D], F32, tag="ps")
        nc.tensor.matmul(pt[:64, :P], src[:], ident[:], is_transpose=True)
        nc.scalar.copy(scnT[:64, t * P : (t + 1) * P], pt[:64, :P])
        m8 = small.tile([P, 8], F32, tag="m8k")
        nc.vector.max(m8[:], src[:])
        mi = small.tile([P, 8], U16, tag="mik")
        nc.vector.max_index(mi[:], m8[:], src[:])
        nc.vector.tensor_copy(mode_cols[:, t : t + 1], mi[:, 0:1])
    # key mode row [1, 1024] -> broadcast -> one-hot [65, 1024] bf16
    ptm = psm.tile([P, D], F32, tag="ps")
    nc.tensor.matmul(ptm[:8, :P], mode_cols[:], ident[:], is_transpose=True)
    mrow8 = singles.tile([8, P], F32)
    nc.scalar.copy(mrow8[:], ptm[:8, :P])
    mode_row = singles.tile([1, S], F32)
    nc.sync.dma_start(mode_row[:].rearrange('a (b c) -> a b c', b=8), mrow8[:])
    mode_row_bf = singles.tile([1, S], BF16)
    nc.vector.tensor_copy(mode_row_bf[:], mode_row[:])
    mode_bc_ps = pz.tile([C, S], F32, tag="z")
    for h in range(2):
        nc.tensor.matmul(mode_bc_ps[:, h * 512 : (h + 1) * 512],
                         ones_row_bf[0:1, :C],
                         mode_row_bf[0:1, h * 512 : (h + 1) * 512],
                         start=True, stop=True, skip_group_check=True)
    ohk = persist.tile([65, S], BF16)
    nc.vector.tensor_scalar(ohk[:64, :], mode_bc_ps[:], mcol[:], None,
                            op0=ALU.is_equal)
    nc.vector.memset(ohk[64:65, :], 1.0)

    # scnT f32r view for the es matmul rhs
    scnT_r = persist.tile([64, S], F32R)
    nc.vector.tensor_copy(scnT_r[:], scnT[:64, :])

    out_dram = io["out"]

    # ---------------- per query tile ----------------
    for qt in [q for _ in range(repeat) for q in range(NQT)]:
        qsl = slice(qt * P, (qt + 1) * P)

        # -- loads --
        hid = work.tile([P, D], F32, tag="hid")
        nc.sync.dma_start(hid[:], io["hidden_q"][qsl, :])
        gv = work.tile([P, 64], F32, tag="gv")
        nc.sync.dma_start(gv[:], io["gv_q"][qsl, :])
        scnq = work.tile([P, 64], F32, tag="scnq")
        nc.sync.dma_start(scnq[:], io["scn_q"][qsl, :])
        mask_bf = work.tile([P, S], BF16, tag="mask")
        nc.sync.dma_start(mask_bf[:], io["maskneg_q"][qsl, :])
        ent = small.tile([P, 1], F32, tag="ent")
        nc.sync.dma_start(ent[:], io["ent_q"][qsl, :])
        conf = small.tile([P, 1], F32, tag="conf")
        nc.sync.dma_start(conf[:], io["conf_q"][qsl, :])
        pkcol = small.tile([C, 1], F32, tag="pkcol")
        nc.sync.dma_start(pkcol[:], io["pk_col"][qt, :, :])
        pk_w = small.tile([P, C // 16], U16, tag="pkw")
        nc.sync.dma_start(pk_w[:], io["pk_wrap"][qt, :, :])
        pq_w = small.tile([P, C // 16], U16, tag="pqw")
        nc.sync.dma_start(pq_w[:], io["pq_wrap"][qt, :, :])
        w4i = small.tile([P, 16], I16, tag="w4i")
        nc.sync.dma_start(w4i[:], io["w4_idx"][qt, :, :])
        s_oh = small.tile([C, P], F32, tag="soh")
        nc.sync.dma_start(s_oh[:], io["s_oh"][qt, :, :])
        k_oh = small.tile([C, 8], F32, tag="koh")
        nc.sync.dma_start(k_oh[:], io["k_oh"][qt, :, :])
        cov = small.tile([P, 1], F32, tag="cov")
        nc.sync.dma_start(cov[:], io["coverage"][qt, :, :])

        # -- hiddenT (f32r) --
        hidT = work.tile([P, 2 * P], F32R, tag="hidT")
        for j in range(2):
            pt = psm.tile([P, D], F32, tag="ps")
            nc.tensor.matmul(pt[:, :P], hid[:, j * P : (j + 1) * P], ident[:],
                             is_transpose=True)
            nc.vector.tensor_copy(hidT[:, j * P : (j + 1) * P],
                                  pt[:, :P].bitcast(F32R))

        # -- gw = sigmoid(hidden . W_gate + b_gate) * conf --
        gscr = work.tile([P, D], F32, tag="gscr")
        gacc = small.tile([P, 1], F32, tag="gacc")
        nc.vector.scalar_tensor_tensor(gscr[:], hid[:], 1.0, wgate_bc[:],
                                       op0=ALU.mult, op1=ALU.mult,
                                       accum_out=gacc[:])
        gw = small.tile([P, 1], F32, tag="gw")
        nc.scalar.activation(gw[:], gacc[:], ACTF.Sigmoid, bias=b_gate[:])
        nc.vector.tensor_tensor(gw[:], gw[:], conf[:], op=ALU.mult)
        one_m_gw = small.tile([P, 1], F32, tag="omg")
        nc.vector.tensor_scalar(one_m_gw[:], gw[:], -1.0, 1.0, op0=ALU.mult,
                                op1=ALU.add)
        aw = small.tile([P, 1], F32, tag="aw")
        nc.vector.tensor_tensor(aw[:], one_m_gw[:], cov[:], op=ALU.mult)
        bw = small.tile([P, 1], F32, tag="bw")
        nc.vector.tensor_scalar(bw[:], cov[:], -1.0, 1.0, op0=ALU.mult, op1=ALU.add)
        nc.vector.tensor_tensor(bw[:], one_m_gw[:], bw[:], op=ALU.mult)

        # -- endpoint pre + squared norm --
        ep = work.tile([P, 64], F32, tag="ep")
        nc.vector.scalar_tensor_tensor(ep[:], gv[:], 0.4, scnq[:],
                                       op0=ALU.mult, op1=ALU.add)
        sq_scr = work.tile([P, D], F32, tag="sqscr")
        ssq2 = small.tile([P, 2], F32, tag="ssq2")
        nc.scalar.activation(sq_scr[:, :64], ep[:], ACTF.Square,
                             accum_out=ssq2[:, 0:1])

        # -- h_pre = hidden + 0.3 * (gv @ W_vel^T), fp32 --
        gvT = work.tile([64, P], F32, tag="gvT")
        ptg = psm.tile([P, D], F32, tag="ps")
        nc.tensor.matmul(ptg[:64, :P], gv[:], ident[:], is_transpose=True)
        nc.scalar.copy(gvT[:], ptg[:64, :P])
        pvel = psm.tile([P, D], F32, tag="ps")
        nc.tensor.matmul(pvel[:], gvT[:], w_velT[:], start=True, stop=True)
        hpre = work.tile([P, D], F32, tag="hpre")
        nc.vector.scalar_tensor_tensor(hpre[:], pvel[:], 0.3, hid[:],
                                       op0=ALU.mult, op1=ALU.add)
        nc.scalar.activation(sq_scr[:], hpre[:], ACTF.Square,
                             accum_out=ssq2[:, 1:2])

        rsq2 = _rsqrt(nc, nc.vector, nwt, ssq2, 2)
        esc = small.tile([P, 1], F32, tag="esc")
        nc.vector.tensor_scalar(esc[:], ent[:], 5.0 / LOGV, None, op0=ALU.mult)
        nc.vector.tensor_tensor(esc[:], esc[:], rsq2[:, 0:1], op=ALU.mult)
        nc.vector.tensor_scalar_mul(ep[:], ep[:], esc[:])
        hsc = small.tile([P, 1], F32, tag="hsc")
        nc.vector.tensor_scalar(hsc[:], rsq2[:, 1:2], 2.5, None, op0=ALU.mult)
        nc.vector.tensor_scalar_mul(hpre[:], hpre[:], hsc[:])

        # -- transposes: epT (f32r), haT (fp32), scn_qT raw + x5 --
        epT = work.tile([64, P], F32R, tag="epT")
        pte = psm.tile([P, D], F32, tag="ps")
        nc.tensor.matmul(pte[:64, :P], ep[:], ident[:], is_transpose=True)
        nc.vector.tensor_copy(epT[:], pte[:64, :P].bitcast(F32R))
        haT = work.tile([P, 2 * P], F32, tag="haT")
        for j in range(2):
            pt = psm.tile([P, D], F32, tag="ps")
            nc.tensor.matmul(pt[:, :P], hpre[:, j * P : (j + 1) * P], ident[:],
                             is_transpose=True)
            nc.scalar.copy(haT[:, j * P : (j + 1) * P], pt[:, :P])
        sqT = work.tile([P, P], F32, tag="sqT")   # raw scn_qT (rows 64+ zero)
        nc.vector.memset(sqT[64:, :], 0.0)
        pts = psm.tile([P, D], F32, tag="ps")
        nc.tensor.matmul(pts[:64, :P], scnq[:], ident[:], is_transpose=True)
        nc.scalar.copy(sqT[:64, :], pts[:64, :P])
        sq5 = work.tile([64, P], F32, tag="sq5")
        nc.vector.tensor_scalar(sq5[:], sqT[:64, :], 5.0, None, op0=ALU.mult)

        # -- query mode one-hot [65, 128] bf16, scaled --
        m8q = small.tile([P, 8], F32, tag="m8q")
        nc.vector.max(m8q[:], scnq[:])
        miq = small.tile([P, 8], U16, tag="miq")
        nc.vector.max_index(miq[:], m8q[:], scnq[:])
        mqf = small.tile([P, 1], F32, tag="mqf")
        nc.vector.tensor_copy(mqf[:], miq[:, 0:1])
        ptq = psm.tile([P, D], F32, tag="ps")
        nc.tensor.matmul(ptq[:1, :P], mqf[:], ident[:], is_transpose=True)
        mrow_q = small.tile([1, P], BF16, tag="mrowq")
        nc.scalar.copy(mrow_q[:], ptq[:1, :P])
        mbq_ps = psm.tile([65, P], F32, tag="ps")
        nc.tensor.matmul(mbq_ps[:], ones_row_bf[0:1, :65], mrow_q[0:1, :],
                         start=True, stop=True, skip_group_check=True)
        ohq = work.tile([65, P], BF16, tag="ohq")
        nc.vector.tensor_scalar(ohq[:], mbq_ps[:], mcol65[:], ohq_scale[:],
                                op0=ALU.is_equal, op1=ALU.mult)
        nc.vector.memset(ohq[64:65, :], -BIGNEG)

        # ---- z_g PSUM: es(f32r) + hm(fp32) + mask(bf16) ----
        zg = pz.tile([P, S], F32, tag="z")
        for h in range(2):
            hs = slice(h * 512, (h + 1) * 512)
            nc.tensor.matmul(zg[:, hs], epT[:], scnT_r[:, hs],
                             start=True, stop=False, skip_group_check=True)
        for j in range(2):
            for h in range(2):
                hs = slice(h * 512, (h + 1) * 512)
                nc.tensor.matmul(zg[:, hs], haT[:, j * P : (j + 1) * P],
                                 msgsT[j][:, hs], start=False, stop=False,
                                 skip_group_check=True)
        for h in range(2):
            hs = slice(h * 512, (h + 1) * 512)
            nc.tensor.matmul(zg[:, hs], ident_bf[:], mask_bf[:, hs],
                             start=False, stop=True, skip_group_check=True)

        # ---- z_l PSUM: 5*scn_sim(fp32) + BIGNEG*(same-1) + mask ----
        zl = pz.tile([P, S], F32, tag="z")
        for h in range(2):
            hs = slice(h * 512, (h + 1) * 512)
            nc.tensor.matmul(zl[:, hs], sq5[:], scnT[:64, hs],
                             start=True, stop=False, skip_group_check=True)
        for h in range(2):
            hs = slice(h * 512, (h + 1) * 512)
            nc.tensor.matmul(zl[:, hs], ohq[:], ohk[:, hs],
                             start=False, stop=False, skip_group_check=True)
        for h in range(2):
            hs = slice(h * 512, (h + 1) * 512)
            nc.tensor.matmul(zl[:, hs], ident_bf[:], mask_bf[:, hs],
                             start=False, stop=True, skip_group_check=True)

        # ---- tau via top-8, both branches batched [P, 2, 8] ----
        t8 = small.tile([P, 16], F32, tag="t8")
        nc.vector.max(t8[:, 0:8], zg[:])
        nc.vector.max(t8[:, 8:16], zl[:])
        c8 = small.tile([P, 16], F32, tag="c8")
        d8 = small.tile([P, 16], F32, tag="d8")
        t8v = t8[:].rearrange("p (b k) -> p b k", b=2)
        c8v = c8[:].rearrange("p (b k) -> p b k", b=2)
        d8v = d8[:].rearrange("p (b k) -> p b k", b=2)
        nc.vector.tensor_copy(c8v[:, :, 0:1], t8v[:, :, 0:1])
        nc.vector.tensor_tensor(c8v[:, :, 1:8], t8v[:, :, 1:8], t8v[:, :, 0:7],
                                op=ALU.add)
        nc.vector.tensor_copy(d8v[:, :, 0:2], c8v[:, :, 0:2])
        nc.vector.tensor_tensor(d8v[:, :, 2:8], c8v[:, :, 2:8], c8v[:, :, 0:6],
                                op=ALU.add)
        nc.vector.tensor_copy(c8v[:, :, 0:4], d8v[:, :, 0:4])
        nc.vector.tensor_tensor(c8v[:, :, 4:8], d8v[:, :, 4:8], d8v[:, :, 0:4],
                                op=ALU.add)
        nc.vector.tensor_scalar(c8[:], c8[:], -1.0, None, op0=ALU.add)
        nc.vector.tensor_tensor(c8v[:], c8v[:],
                                rk8[:, :].unsqueeze(1).broadcast_to([P, 2, 8]),
                                op=ALU.mult)
        tau2 = small.tile([P, 2], F32, tag="tau2")
        nc.vector.tensor_reduce(tau2[:], c8v[:], axis=AXX, op=ALU.max)
        tau_g = tau2[:, 0:1]
        tau_l = tau2[:, 1:2]
        if STAGE < 2:
            outt = work.tile([P, D], F32, tag="outt")
            nc.vector.tensor_scalar_mul(outt[:], hid[:], tau_g[:])
            nc.vector.tensor_scalar_mul(outt[:], outt[:], tau_l[:])
            nc.sync.dma_start(out_dram[qsl, :], outt[:])
            continue

        # ---- W1' = relu(gw*zg - gw*tau_g), W2' = relu(bw*zl - bw*tau_l) ----
        nbias_g = small.tile([P, 1], F32, tag="nbg")
        nc.vector.tensor_tensor(nbias_g[:], gw[:], tau_g, op=ALU.mult)
        nc.vector.tensor_scalar(nbias_g[:], nbias_g[:], -1.0, None, op0=ALU.mult)
        nbias_l = small.tile([P, 1], F32, tag="nbl")
        nc.vector.tensor_tensor(nbias_l[:], bw[:], tau_l, op=ALU.mult)
        nc.vector.tensor_scalar(nbias_l[:], nbias_l[:], -1.0, None, op0=ALU.mult)
        w1 = work.tile([P, S], F32R, tag="w1")
        nc.scalar.activation(w1[:], zg[:], ACTF.Relu, bias=nbias_g[:], scale=gw[:])
        w2 = work.tile([P, S], F32R, tag="w2")
        nc.scalar.activation(w2[:], zl[:], ACTF.Relu, bias=nbias_l[:], scale=bw[:])

        if STAGE < 3:
            outt = work.tile([P, D], F32, tag="outt")
            nc.scalar.activation(outt[:], w1[:].bitcast(F32)[:, 0:D], ACTF.Copy)
            nc.vector.tensor_tensor(outt[:], outt[:], w2[:].bitcast(F32)[:, 0:D], op=ALU.add)
            nc.sync.dma_start(out_dram[qsl, :], outt[:])
            continue
        # ---- static branch ----
        g1 = work.tile([P, C], F32, tag="g1")
        nc.gpsimd.indirect_copy(g1[:], scnT[:, :], pk_w[:], True)
        g2 = work.tile([P, C], F32, tag="g2")
        nc.gpsimd.indirect_copy(g2[:], sqT[:, :], pq_w[:], True)
        pp = work.tile([C, C], F32, tag="pp")
        nc.vector.tensor_tensor(pp[:64, :], g1[:64, :], g2[:64, :], op=ALU.mult)
        ptsc = psm.tile([P, D], F32, tag="ps")
        nc.tensor.matmul(ptsc[:C, 0:1], pp[:64, :], ones64[:],
                         start=True, stop=True)
        simc = small.tile([C, 1], F32, tag="simc")
        nc.scalar.copy(simc[:], ptsc[:C, 0:1])
        krhs = small.tile([C, 8], F32, tag="krhs")
        nc.vector.tensor_scalar_mul(krhs[:], k_oh[:], simc[:])
        pss = psm.tile([P, D], F32, tag="ps")
        nc.tensor.matmul(pss[:, 0:8], s_oh[:], krhs[:], start=True, stop=True)
        sim_s = small.tile([P, 8], F32, tag="sims")
        nc.scalar.copy(sim_s[:], pss[:, 0:8])

        # edge MLP (batched): x2 = 0.5*(sim*W_e1[h] + b_e1[h]), h-major [P,64]
        x2 = work.tile([P, 64], F32, tag="x2")
        x2v = x2[:].rearrange("p (h k) -> p h k", h=8)
        sim_bc = sim_s[:].unsqueeze(1).broadcast_to([P, 8, 8])
        w1v = w_e1[:, :].rearrange("p (h k) -> p h k", h=8)
        b1v = b_e1[:, :].rearrange("p (h k) -> p h k", h=8)
        nc.vector.tensor_tensor(x2v, sim_bc, w1v, op=ALU.mult)
        nc.vector.tensor_tensor(x2v, x2v, b1v, op=ALU.add)
        # tanh-gelu: gelu(x) ~= x2*(1+tanh(c*(2*x2)*(1+0.17886*x2^2))), x2=x/2
        xsq = work.tile([P, 64], F32, tag="xsq")
        nc.vector.tensor_tensor(xsq[:], x2[:], x2[:], op=ALU.mult)
        nc.vector.tensor_scalar(xsq[:], xsq[:], 0.35772, 2.0, op0=ALU.mult,
                                op1=ALU.add)
        nc.vector.tensor_tensor(xsq[:], xsq[:], x2[:], op=ALU.mult)
        erf = work.tile([P, 64], F32, tag="erf")
        nc.scalar.activation(erf[:], xsq[:], ACTF.Tanh, scale=0.7978845608028654)
        he = work.tile([P, 64], F32, tag="he")
        nc.vector.scalar_tensor_tensor(he[:], erf[:], 1.0, x2[:],
                                       op0=ALU.add, op1=ALU.mult)
        nc.vector.tensor_tensor(he[:], he[:], w_e2[:, :], op=ALU.mult)
        edge = small.tile([P, 8], F32, tag="edge")
        nc.vector.tensor_reduce(
            edge[:], he[:].rearrange("p (h k) -> p k h", h=8),
            axis=AXX, op=ALU.add)
        nc.vector.tensor_scalar(edge[:], edge[:], b_e2[:], None, op0=ALU.add)
        # softmax over k via sigmoid-exp identity
        mx = small.tile([P, 1], F32, tag="mx")
        nc.vector.tensor_reduce(mx[:], edge[:], axis=AXX, op=ALU.max)
        sg = small.tile([P, 8], F32, tag="sg")
        nc.scalar.activation(sg[:], edge[:], ACTF.Sigmoid, bias=mx[:], scale=-1.0)
        ex = small.tile([P, 8], F32, tag="ex")
        nc.vector.reciprocal(ex[:], sg[:])
        nc.vector.tensor_scalar(ex[:], ex[:], -1.0, None, op0=ALU.add)
        den = small.tile([P, 1], F32, tag="den")
        nc.vector.tensor_reduce(den[:], ex[:], axis=AXX, op=ALU.add)
        rden = small.tile([P, 1], F32, tag="rden")
        nc.vector.reciprocal(rden[:], den[:])
        ews = small.tile([P, 8], F32, tag="ews")
        nc.vector.tensor_scalar_mul(ews[:], ex[:], rden[:])
        # scatter weights: w4d layout slot-major [si*8 + k]
        w4d = small.tile([P, 16], BF16, tag="w4d")
        ewsaw = small.tile([P, 8], F32, tag="ewsaw")
        nc.vector.tensor_scalar_mul(ewsaw[:], ews[:], aw[:])
        nc.vector.tensor_copy(w4d[:, 0:8], ewsaw[:])
        nc.vector.tensor_copy(w4d[:, 8:16], ewsaw[:])
        w4 = work.tile([P, C], BF16, tag="w4")
        nc.gpsimd.local_scatter(w4[:], w4d[:], w4i[:], channels=P,
                                num_elems=C, num_idxs=16)
        ptw = psm.tile([P, D], BF16, tag="ps")
        nc.tensor.matmul(ptw[:C, :P], w4[:], ident_bf[:], is_transpose=True)
        w4T = small.tile([C, P], BF16, tag="w4T")
        nc.vector.tensor_copy(w4T[:], ptw[:C, :P])
        ohp = work.tile([C, S], BF16, tag="ohp")
        nc.vector.tensor_scalar(ohp[:], iota16[:], pkcol[:], None, op0=ALU.is_equal)

        if STAGE < 4:
            outt = work.tile([P, D], F32, tag="outt")
            nc.vector.tensor_scalar_mul(outt[:], hid[:], aw[:])
            nc.vector.tensor_tensor(outt[:, 0:8], outt[:, 0:8], ews[:], op=ALU.add)
            nc.vector.tensor_tensor(outt[:, 0:64], outt[:, 0:64], w4[:].bitcast(F32)[:, 0:32].bitcast(BF16), op=ALU.add)
            nc.sync.dma_start(out_dram[qsl, :], outt[:])
            continue
        # ---- WcT accumulation: W1'^T + W2'^T + static oh-matmul ----
        wcT_ps = pz.tile([P, S], F32, tag="z")
        for i in range(8):
            sl = slice(i * P, (i + 1) * P)
            nc.tensor.matmul(wcT_ps[:, sl].bitcast(F32R), w1[:, sl], ident_r[:],
                             is_transpose=True, start=True, stop=False,
                             skip_group_check=True)
            nc.tensor.matmul(wcT_ps[:, sl].bitcast(F32R), w2[:, sl], ident_r[:],
                             is_transpose=True, start=False, stop=False,
                             skip_group_check=True)
            nc.tensor.matmul(wcT_ps[:, sl], ohp[:, sl], w4T[:],
                             start=False, stop=True, skip_group_check=True)
        wcT = work.tile([P, S], F32R, tag="wcTs")
        nc.scalar.copy(wcT[:], wcT_ps[:].bitcast(F32R))

        if STAGE < 5:
            outt = work.tile([P, D], F32, tag="outt")
            nc.scalar.activation(outt[:], wcT[:].bitcast(F32)[:, 0:D], ACTF.Copy)
            nc.sync.dma_start(out_dram[qsl, :], outt[:])
            continue
        # ---- agg (f32r) and probe (f32r) ----
        agg = psm.tile([P, D], F32, tag="ps")
        for i in range(8):
            nc.tensor.matmul(agg[:], wcT[:, i * P : (i + 1) * P],
                             msgs_r[:, i * D : (i + 1) * D],
                             start=(i == 0), stop=(i == 7))
        probe = psm.tile([P, D], F32, tag="ps")
        for j in range(2):
            nc.tensor.matmul(probe[:], hidT[:, j * P : (j + 1) * P],
                             w_probeT[j][:], start=(j == 0), stop=(j == 1))

        # ---- rel = sigmoid(dot * rsqrt(na2*np2)); out = agg * rel ----
        probe_sb = work.tile([P, D], F32, tag="prsb")
        nc.scalar.copy(probe_sb[:], probe[:])
        na2 = small.tile([P, 1], F32, tag="na2")
        sq_scr2 = work.tile([P, D], F32, tag="sqscr2")
        nc.scalar.activation(sq_scr2[:], agg[:], ACTF.Square, accum_out=na2[:])
        np2 = small.tile([P, 1], F32, tag="np2")
        sq_scr3 = work.tile([P, D], F32, tag="sqscr3")
        nc.scalar.activation(sq_scr3[:], probe_sb[:], ACTF.Square, accum_out=np2[:])
        dot = small.tile([P, 1], F32, tag="dot")
        dscr = work.tile([P, D], F32, tag="dscr")
        nc.vector.scalar_tensor_tensor(dscr[:], agg[:], 1.0, probe_sb[:],
                                       op0=ALU.mult, op1=ALU.mult,
                                       accum_out=dot[:])
        nn2 = small.tile([P, 1], F32, tag="nn2")
        nc.vector.tensor_tensor(nn2[:], na2[:], np2[:], op=ALU.mult)
        rsqn = _rsqrt(nc, nc.vector, nwt, nn2, 1)
        rel = small.tile([P, 1], F32, tag="rel")
        nc.vector.tensor_tensor(rel[:], dot[:], rsqn[:], op=ALU.mult)
        nc.scalar.activation(rel[:], rel[:], ACTF.Sigmoid)
        outt = work.tile([P, D], F32, tag="outt")
        nc.vector.tensor_scalar_mul(outt[:], agg[:], rel[:])
        nc.sync.dma_start(out_dram[qsl, :], outt[:])


# ------------------------------------------------------------------ host ----

_CACHE = {}


def _build_module(repeat: int = 1):
    key = f"nc{repeat}"
    if key in _CACHE:
        return _CACHE[key]
    nc = bacc.Bacc("TRN2", target_bir_lowering=False, debug=False, num_devices=8)

    def dt_(name, shape, dtype, kind="ExternalInput"):
        return nc.dram_tensor(name, shape, dtype, kind=kind).ap()

    io = {
        "messages": dt_("messages", [S, D], F32),
        "scn": dt_("scn", [S, 64], F32),
        "scn_q": dt_("scn_q", [Q, 64], F32),
        "hidden_q": dt_("hidden_q", [Q, D], F32),
        "gv_q": dt_("gv_q", [Q, 64], F32),
        "maskneg_q": dt_("maskneg_q", [Q, S], BF16),
        "ent_q": dt_("ent_q", [Q, 1], F32),
        "conf_q": dt_("conf_q", [Q, 1], F32),
        "W_vel": dt_("W_vel", [D, NS], F32),
        "W_probe": dt_("W_probe", [D, D], F32),
        "W_gate": dt_("W_gate", [1, D], F32),
        "W_e1c": dt_("W_e1c", [1, 64], F32),
        "b_e1c": dt_("b_e1c", [1, 64], F32),
        "W_e2c": dt_("W_e2c", [1, 64], F32),
        "b_e2c": dt_("b_e2c", [1, 1], F32),
        "b_gatec": dt_("b_gatec", [1, 1], F32),
        "rk8c": dt_("rk8c", [1, 8], F32),
        "iota16c": dt_("iota16c", [1, S], F32),
        "mcolc": dt_("mcolc", [C, 1], F32),
        "mcol65c": dt_("mcol65c", [65, 1], F32),
        "ohqscalec": dt_("ohqscalec", [65, 1], F32),
        "pk_col": dt_("pk_col", [NQT, C, 1], F32),
        "pk_wrap": dt_("pk_wrap", [NQT, 128, C // 16], U16),
        "pq_wrap": dt_("pq_wrap", [NQT, 128, C // 16], U16),
        "w4_idx": dt_("w4_idx", [NQT, 128, 16], I16),
        "s_oh": dt_("s_oh", [NQT, C, 128], F32),
        "k_oh": dt_("k_oh", [NQT, C, 8], F32),
        "coverage": dt_("coverage", [NQT, 128, 1], F32),
        "out": dt_("out", [Q, D], F32, kind="ExternalOutput"),
    }
    with tile.TileContext(nc) as tc:
        build_kernel(tc, io, repeat)
    nc.compile()
    _CACHE[key] = nc
    return nc


def _host_prep(inputs):
    import ml_dtypes
    msgs = np.ascontiguousarray(inputs["messages"], dtype=np.float32)
    hid = np.ascontiguousarray(inputs["hidden"], dtype=np.float32)
    x_ids = np.asarray(inputs["x_ids"]).astype(np.int64)
    scn = np.ascontiguousarray(inputs["scn"], dtype=np.float32)
    mask = np.asarray(inputs["mask"]).astype(bool)
    static_nb = np.asarray(inputs["static_nb"]).astype(np.int64)
    gv = np.ascontiguousarray(inputs["geo_velocity"], dtype=np.float32)
    conf = np.asarray(inputs["ctx_conf"]).astype(np.float32).reshape(B, S, 1)
    ent = np.asarray(inputs["current_entropy"]).astype(np.float32)

    blocked = mask | np.eye(S, dtype=bool)
    maskneg = (MASKNEG * blocked.astype(np.float32)).astype(ml_dtypes.bfloat16)

    rk8 = (1.0 / np.arange(1, 9, dtype=np.float32)).reshape(1, 8)
    iota16 = np.arange(S, dtype=np.float32).reshape(1, S)
    mcol = np.arange(C, dtype=np.float32).reshape(C, 1)
    mcol65 = np.full((65, 1), -1.0, np.float32)
    mcol65[:64, 0] = np.arange(64)
    ohqsc = np.full((65, 1), BIGNEG, np.float32)
    ohqsc[64, 0] = -BIGNEG

    shared = {
        "W_vel": np.ascontiguousarray(inputs["W_vel"], dtype=np.float32),
        "W_probe": np.ascontiguousarray(inputs["W_probe"], dtype=np.float32),
        "W_gate": np.ascontiguousarray(inputs["W_gate"], dtype=np.float32).reshape(1, D),
        "W_e1c": np.repeat(0.5 * np.asarray(inputs["W_e1"], np.float32).reshape(8), 8).reshape(1, 64),
        "b_e1c": np.repeat(0.5 * np.asarray(inputs["b_e1"], np.float32).reshape(8), 8).reshape(1, 64),
        "W_e2c": np.repeat(np.asarray(inputs["W_e2"], np.float32).reshape(8), 8).reshape(1, 64),
        "b_e2c": np.asarray(inputs["b_e2"], np.float32).reshape(1, 1),
        "b_gatec": np.asarray(inputs["b_gate"], np.float32).reshape(1, 1),
        "rk8c": rk8, "iota16c": iota16, "mcolc": mcol, "mcol65c": mcol65,
        "ohqscalec": ohqsc,
    }

    in_maps = []
    for core in range(8):
        b, half = divmod(core, 2)
        base = half * Q
        pos = {}
        for t, v in enumerate(x_ids[b]):
            pos.setdefault(int(v), []).append(t)
        nbv = static_nb[x_ids[b, base : base + Q]]

        pk_col = np.zeros((NQT, C, 1), np.float32)
        pk_wrap = np.zeros((NQT, 128, C // 16), np.uint16)
        pq_wrap = np.zeros((NQT, 128, C // 16), np.uint16)
        w4_idx = np.full((NQT, 128, 16), -1, np.int16)
        s_oh = np.zeros((NQT, C, 128), np.float32)
        k_oh = np.zeros((NQT, C, 8), np.float32)
        coverage = np.zeros((NQT, 128, 1), np.float32)

        for qt in range(NQT):
            pairs = []
            cnt = np.zeros((128, 8), np.float32)
            for s_loc in range(128):
                s_glob = base + qt * 128 + s_loc
                for k in range(8):
                    v = int(nbv[qt * 128 + s_loc, k])
                    ms = [t for t in pos.get(v, []) if t <= s_glob]
                    cnt[s_loc, k] = len(ms)
                    assert len(ms) <= 2, f"cnt>2: core{core} qt{qt}"
                    for si, t in enumerate(ms):
                        pairs.append((s_loc, k, t, si))
            assert len(pairs) <= C, f"{len(pairs)} pairs > C: core{core} qt{qt}"
            flatk = np.zeros(C, np.uint16)
            flatq = np.zeros(C, np.uint16)
            for j, (s_loc, k, t, si) in enumerate(pairs):
                pk_col[qt, j, 0] = t
                s_oh[qt, j, s_loc] = 1.0
                k_oh[qt, j, k] = 1.0
                w4_idx[qt, s_loc, si * 8 + k] = j
                flatk[j] = t
                flatq[j] = s_loc
            for p in range(128):
                for sw in range(C // 16):
                    pk_wrap[qt, p, sw] = flatk[sw * 16 + (p % 16)]
                    pq_wrap[qt, p, sw] = flatq[sw * 16 + (p % 16)]
            coverage[qt, :, 0] = np.minimum(cnt, 1.0).mean(-1)
            # unmatched pairs -> pk_col must not accidentally one-hot-match:
            for j in range(len(pairs), C):
                pk_col[qt, j, 0] = -1

        in_maps.append({
            **shared,
            "messages": msgs[b],
            "scn": scn[b],
            "scn_q": scn[b, base : base + Q],
            "hidden_q": hid[b, base : base + Q],
            "gv_q": gv[b, base : base + Q],
            "maskneg_q": np.ascontiguousarray(maskneg[base : base + Q]),
            "ent_q": ent[b, base : base + Q].reshape(Q, 1),
            "conf_q": conf[b, base : base + Q].reshape(Q, 1),
            "pk_col": pk_col, "pk_wrap": pk_wrap, "pq_wrap": pq_wrap,
            "w4_idx": w4_idx, "s_oh": s_oh, "k_oh": k_oh, "coverage": coverage,
        })
    return in_maps


def run(inputs, trace=False, repeat=1):
    in_maps = _host_prep(inputs)
    nc = _build_module(repeat)
    br = run_bass_kernel_spmd(nc, in_maps, list(range(8)), trace=trace)
    out = np.zeros((B, S, D), np.float32)
    for core in range(8):
        b, half = divmod(core, 2)
        out[b, half * Q : (half + 1) * Q] = br.results[core]["out"]
    return out, br


def kernel(**inputs):
    out, _ = run(inputs)
    return out


# revision 27
# speedup vs baseline: 1.2543x; 1.0213x over previous
"""Trainium2 Bass kernel for nn_EntropyGeoRouter.

Sharding: 8 cores; core c handles batch b=c//2, sequence-half h=c%2
(512 query rows, full 1024 keys of that batch). 4 query tiles of 128 rows.
One SPMD module for all cores; per-core differences live in input data
(host passes only index-derived tensors; all float math runs on device).
"""
import math
import os
import numpy as np
from contextlib import ExitStack

import concourse.bass as bass
import concourse.bacc as bacc
import concourse.tile as tile
from concourse import mybir, masks
from concourse._compat import with_exitstack
from concourse.bass_utils import run_bass_kernel_spmd

F32 = mybir.dt.float32
F32R = mybir.dt.float32r
BF16 = mybir.dt.bfloat16
I16 = mybir.dt.int16
U16 = mybir.dt.uint16
I32 = mybir.dt.int32
ALU = mybir.AluOpType
ACTF = mybir.ActivationFunctionType
AXX = mybir.AxisListType.X

B, S, D, NS, KNB, VOCAB = 4, 1024, 256, 64, 8, 32000
BIGNEG = 30000.0      # same-mode additive gate
MASKNEG = -50000.0    # blocked additive (mask|eye), pre-scaled for z*5
Q = 512               # query rows per core
NQT = 4               # query tiles per core
P = 128
C = 64                # match-pair capacity per query tile
LOGV = math.log(VOCAB)
RSQRT_MAGIC = 0x5F3759DF
STAGE = int(os.environ.get('KSTAGE', '5'))


def _rsqrt(nc, eng, pool, x, ncols):
    """rsqrt of positive [128, ncols] f32 via bit trick + 2 Newton steps."""
    y = pool.tile([P, ncols], F32, tag="nt_y")
    t = pool.tile([P, ncols], F32, tag="nt_t")
    u = pool.tile([P, ncols], F32, tag="nt_u")
    yi = y[:, :].bitcast(I32)
    xi = x[:, :].bitcast(I32)
    eng.tensor_scalar(yi, xi, 1, None, op0=ALU.arith_shift_right)
    eng.tensor_scalar(yi, yi, -1, RSQRT_MAGIC, op0=ALU.mult, op1=ALU.add)
    for _ in range(2):
        eng.tensor_tensor(t[:, :], y[:, :], y[:, :], op=ALU.mult)
        eng.tensor_tensor(u[:, :], t[:, :], x[:, :], op=ALU.mult)
        eng.tensor_scalar(u[:, :], u[:, :], -0.5, 1.5, op0=ALU.mult, op1=ALU.add)
        eng.tensor_tensor(y[:, :], y[:, :], u[:, :], op=ALU.mult)
    return y


@with_exitstack
def build_kernel(ctx: ExitStack, tc: tile.TileContext, io: dict, repeat: int = 1):
    nc = tc.nc

    singles = ctx.enter_context(tc.tile_pool(name="singles", bufs=1))
    persist = ctx.enter_context(tc.tile_pool(name="persist", bufs=1))
    work = ctx.enter_context(tc.tile_pool(name="work", bufs=4))
    small = ctx.enter_context(tc.tile_pool(name="small", bufs=4))
    nwt = ctx.enter_context(tc.tile_pool(name="newton", bufs=4))
    pz = ctx.enter_context(tc.tile_pool(name="pz", bufs=2, space="PSUM"))
    psm = ctx.enter_context(tc.tile_pool(name="psm", bufs=4, space="PSUM"))

    # --- identities ---
    ident = singles.tile([P, P], F32)
    masks.make_identity(nc, ident[:])
    ident_r = singles.tile([P, P], F32R)
    nc.vector.tensor_copy(ident_r[:], ident[:])
    ident_bf = singles.tile([P, P], BF16)
    masks.make_identity(nc, ident_bf[:])

    # --- broadcast params / constants ---
    def bcast(name, cols, dtype=F32, parts=P):
        t = singles.tile([parts, cols], dtype, name="bc_" + name)
        nc.sync.dma_start(t[:], io[name][:].partition_broadcast(parts))
        return t

    w_e1 = bcast("W_e1c", 64)   # h-major, each 0.5*W_e1[h] x8
    b_e1 = bcast("b_e1c", 64)
    w_e2 = bcast("W_e2c", 64)
    b_e2 = bcast("b_e2c", 1)
    b_gate = bcast("b_gatec", 1)
    rk8 = bcast("rk8c", 8)       # [1, 1/2, ..., 1/8]
    iota16 = bcast("iota16c", S, F32, C)   # iota16[j, t] = t
    mcol = singles.tile([C, 1], F32)      # 0..63 column
    nc.sync.dma_start(mcol[:], io["mcolc"][:])
    mcol65 = singles.tile([65, 1], F32)   # 0..63, row64 = -1
    nc.sync.dma_start(mcol65[:], io["mcol65c"][:])
    ohq_scale = singles.tile([65, 1], F32)  # +BIGNEG rows, -BIGNEG row 64
    nc.sync.dma_start(ohq_scale[:], io["ohqscalec"][:])
    ones64 = singles.tile([C, 1], F32)
    nc.vector.memset(ones64[:], 1.0)
    ones_row_bf = singles.tile([1, P], BF16)
    nc.vector.memset(ones_row_bf[:], 1.0)

    # W_gate broadcast row [128, 256]
    wgate_bc = bcast("W_gate", D)

    # W_velT [64, 256] fp32
    w_velT = persist.tile([64, D], F32)
    for i in range(2):
        src = work.tile([P, 64], F32, tag="ld64")
        nc.sync.dma_start(src[:], io["W_vel"][i * P : (i + 1) * P, :])
        pt = psm.tile([P, D], F32, tag="ps")
        nc.tensor.matmul(pt[:64, :P], src[:], ident[:], is_transpose=True)
        nc.scalar.copy(w_velT[:, i * P : (i + 1) * P], pt[:64, :P])

    # W_probeT [256,256] f32r as two [128,256] tiles (w_probeT[j][dT, d'])
    w_probeT = [persist.tile([P, D], F32R, name=f"wpT{i}") for i in range(2)]
    for i in range(2):
        src = work.tile([P, D], F32R, tag="ldr")
        nc.sync.dma_start(src[:], io["W_probe"][i * P : (i + 1) * P, :].bitcast(F32R))
        for j in range(2):
            pt = psm.tile([P, D], F32, tag="ps")
            nc.tensor.matmul(pt[:, :P].bitcast(F32R), src[:, j * P : (j + 1) * P],
                             ident_r[:], is_transpose=True)
            nc.vector.tensor_copy(w_probeT[j][:, i * P : (i + 1) * P],
                                  pt[:, :P].bitcast(F32R))

    # messages f32r copy for agg rhs: [128, 8*256], t-slab major
    msgs_r = persist.tile([P, 8 * D], F32R)
    for t in range(8):
        nc.sync.dma_start(msgs_r[:, t * D : (t + 1) * D],
                          io["messages"][t * P : (t + 1) * P, :].bitcast(F32R))

    # messagesT [256, 1024] fp32 as two [128, 1024] tiles (d-slab major)
    msgsT = [persist.tile([P, S], F32, name=f"msgsT{i}") for i in range(2)]
    for t in range(8):
        src = work.tile([P, D], F32, tag="ldm")
        nc.sync.dma_start(src[:], io["messages"][t * P : (t + 1) * P, :])
        for j in range(2):
            pt = psm.tile([P, D], F32, tag="ps")
            nc.tensor.matmul(pt[:, :P], src[:, j * P : (j + 1) * P], ident[:],
                             is_transpose=True)
            nc.scalar.copy(msgsT[j][:, t * P : (t + 1) * P], pt[:, :P])

    # scnT [128(pad), 1024] fp32 (rows 64..127 zero) + key modes
    scnT = persist.tile([P, S], F32)
    nc.vector.memset(scnT[64:, :], 0.0)
    mode_cols = persist.tile([P, 8], F32)
    for t in range(8):
        src = work.tile([P, 64], F32, tag="ld64")
        nc.sync.dma_start(src[:], io["scn"][t * P : (t + 1) * P, :])
        pt = psm.tile([P, D], F32, tag="ps")
        nc.tensor.matmul(pt[:64, :P], src[:], ident[:], is_transpose=True)
        nc.scalar.copy(scnT[:64, t * P : (t + 1) * P], pt[:64, :P])
        m8 = small.tile([P, 8], F32, tag="m8k")
        nc.vector.max(m8[:], src[:])
        mi = small.tile([P, 8], U16, tag="mik")
        nc.vector.max_index(mi[:], m8[:], src[:])
        nc.vector.tensor_copy(mode_cols[:, t : t + 1], mi[:, 0:1])
    # key mode row [1, 1024] -> broadcast -> one-hot [65, 1024] bf16
    ptm = psm.tile([P, D], F32, tag="ps")
    nc.tensor.matmul(ptm[:8, :P], mode_cols[:], ident[:], is_transpose=True)
    mrow8 = singles.tile([8, P], F32)
    nc.scalar.copy(mrow8[:], ptm[:8, :P])
    mode_row = singles.tile([1, S], F32)
    nc.sync.dma_start(mode_row[:].rearrange('a (b c) -> a b c', b=8), mrow8[:])
    mode_row_bf = singles.tile([1, S], BF16)
    nc.vector.tensor_copy(mode_row_bf[:], mode_row[:])
    mode_bc_ps = pz.tile([C, S], F32, tag="z")
    for h in range(2):
        nc.tensor.matmul(mode_bc_ps[:, h * 512 : (h + 1) * 512],
                         ones_row_bf[0:1, :C],
                         mode_row_bf[0:1, h * 512 : (h + 1) * 512],
                         start=True, stop=True, skip_group_check=True)
    ohk = persist.tile([65, S], BF16)
    nc.vector.tensor_scalar(ohk[:64, :], mode_bc_ps[:], mcol[:], None,
                            op0=ALU.is_equal)
    nc.vector.memset(ohk[64:65, :], 1.0)

    # scnT f32r view for the es matmul rhs
    scnT_r = persist.tile([64, S], F32R)
    nc.vector.tensor_copy(scnT_r[:], scnT[:64, :])

    out_dram = io["out"]

    # ---------------- per query tile ----------------
    for qt in [q for _ in range(repeat) for q in range(NQT)]:
        qsl = slice(qt * P, (qt + 1) * P)

        # -- loads --
        hid = work.tile([P, D], F32, tag="hid")
        nc.sync.dma_start(hid[:], io["hidden_q"][qsl, :])
        gv = work.tile([P, 64], F32, tag="gv")
        nc.sync.dma_start(gv[:], io["gv_q"][qsl, :])
        scnq = work.tile([P, 64], F32, tag="scnq")
        nc.sync.dma_start(scnq[:], io["scn_q"][qsl, :])
        mask_bf = work.tile([P, S], BF16, tag="mask")
        nc.sync.dma_start(mask_bf[:], io["maskneg_q"][qsl, :])
        ent = small.tile([P, 1], F32, tag="ent")
        nc.sync.dma_start(ent[:], io["ent_q"][qsl, :])
        conf = small.tile([P, 1], F32, tag="conf")
        nc.sync.dma_start(conf[:], io["conf_q"][qsl, :])
        pkcol = small.tile([C, 1], F32, tag="pkcol")
        nc.sync.dma_start(pkcol[:], io["pk_col"][qt, :, :])
        pk_w = small.tile([P, C // 16], U16, tag="pkw")
        nc.sync.dma_start(pk_w[:], io["pk_wrap"][qt, :, :])
        pq_w = small.tile([P, C // 16], U16, tag="pqw")
        nc.sync.dma_start(pq_w[:], io["pq_wrap"][qt, :, :])
        w4i = small.tile([P, 16], I16, tag="w4i")
        nc.sync.dma_start(w4i[:], io["w4_idx"][qt, :, :])
        s_oh = small.tile([C, P], F32, tag="soh")
        nc.sync.dma_start(s_oh[:], io["s_oh"][qt, :, :])
        k_oh = small.tile([C, 8], F32, tag="koh")
        nc.sync.dma_start(k_oh[:], io["k_oh"][qt, :, :])
        cov = small.tile([P, 1], F32, tag="cov")
        nc.sync.dma_start(cov[:], io["coverage"][qt, :, :])

        # -- hiddenT (f32r) --
        hidT = work.tile([P, 2 * P], F32R, tag="hidT")
        for j in range(2):
            pt = psm.tile([P, D], F32, tag="ps")
            nc.tensor.matmul(pt[:, :P], hid[:, j * P : (j + 1) * P], ident[:],
                             is_transpose=True)
            nc.vector.tensor_copy(hidT[:, j * P : (j + 1) * P],
                                  pt[:, :P].bitcast(F32R))

        # -- gw = sigmoid(hidden . W_gate + b_gate) * conf --
        gscr = work.tile([P, D], F32, tag="gscr")
        gacc = small.tile([P, 1], F32, tag="gacc")
        nc.vector.scalar_tensor_tensor(gscr[:], hid[:], 1.0, wgate_bc[:],
                                       op0=ALU.mult, op1=ALU.mult,
                                       accum_out=gacc[:])
        gw = small.tile([P, 1], F32, tag="gw")
        nc.scalar.activation(gw[:], gacc[:], ACTF.Sigmoid, bias=b_gate[:])
        nc.vector.tensor_tensor(gw[:], gw[:], conf[:], op=ALU.mult)
        one_m_gw = small.tile([P, 1], F32, tag="omg")
        nc.vector.tensor_scalar(one_m_gw[:], gw[:], -1.0, 1.0, op0=ALU.mult,
                                op1=ALU.add)
        aw = small.tile([P, 1], F32, tag="aw")
        nc.vector.tensor_tensor(aw[:], one_m_gw[:], cov[:], op=ALU.mult)
        bw = small.tile([P, 1], F32, tag="bw")
        nc.vector.tensor_scalar(bw[:], cov[:], -1.0, 1.0, op0=ALU.mult, op1=ALU.add)
        nc.vector.tensor_tensor(bw[:], one_m_gw[:], bw[:], op=ALU.mult)

        sqT = work.tile([P, P], F32, tag="sqT")   # raw scn_qT (rows 64+ zero)
        nc.vector.memset(sqT[64:, :], 0.0)
        pts = psm.tile([P, D], F32, tag="ps")
        nc.tensor.matmul(pts[:64, :P], scnq[:], ident[:], is_transpose=True)
        nc.scalar.copy(sqT[:64, :], pts[:64, :P])
        sq5 = work.tile([64, P], F32, tag="sq5")
        nc.vector.tensor_scalar(sq5[:], sqT[:64, :], 5.0, None, op0=ALU.mult)
        # -- query mode one-hot [65, 128] bf16, scaled --
        m8q = small.tile([P, 8], F32, tag="m8q")
        nc.vector.max(m8q[:], scnq[:])
        miq = small.tile([P, 8], U16, tag="miq")
        nc.vector.max_index(miq[:], m8q[:], scnq[:])
        mqf = small.tile([P, 1], F32, tag="mqf")
        nc.vector.tensor_copy(mqf[:], miq[:, 0:1])
        ptq = psm.tile([P, D], F32, tag="ps")
        nc.tensor.matmul(ptq[:1, :P], mqf[:], ident[:], is_transpose=True)
        mrow_q = small.tile([1, P], BF16, tag="mrowq")
        nc.scalar.copy(mrow_q[:], ptq[:1, :P])
        mbq_ps = psm.tile([65, P], F32, tag="ps")
        nc.tensor.matmul(mbq_ps[:], ones_row_bf[0:1, :65], mrow_q[0:1, :],
                         start=True, stop=True, skip_group_check=True)
        ohq = work.tile([65, P], BF16, tag="ohq")
        nc.vector.tensor_scalar(ohq[:], mbq_ps[:], mcol65[:], ohq_scale[:],
                                op0=ALU.is_equal, op1=ALU.mult)
        nc.vector.memset(ohq[64:65, :], -BIGNEG)

        # ---- z_l PSUM: 5*scn_sim(fp32) + BIGNEG*(same-1) + mask ----
        zl = pz.tile([P, S], F32, tag="z")
        for h in range(2):
            hs = slice(h * 512, (h + 1) * 512)
            nc.tensor.matmul(zl[:, hs], sq5[:], scnT[:64, hs],
                             start=True, stop=False, skip_group_check=True)
        for h in range(2):
            hs = slice(h * 512, (h + 1) * 512)
            nc.tensor.matmul(zl[:, hs], ohq[:], ohk[:, hs],
                             start=False, stop=False, skip_group_check=True)
        for h in range(2):
            hs = slice(h * 512, (h + 1) * 512)
            nc.tensor.matmul(zl[:, hs], ident_bf[:], mask_bf[:, hs],
                             start=False, stop=True, skip_group_check=True)

        # -- endpoint pre + squared norm --
        ep = work.tile([P, 64], F32, tag="ep")
        nc.vector.scalar_tensor_tensor(ep[:], gv[:], 0.4, scnq[:],
                                       op0=ALU.mult, op1=ALU.add)
        sq_scr = work.tile([P, D], F32, tag="sqscr")
        ssq2 = small.tile([P, 2], F32, tag="ssq2")
        nc.scalar.activation(sq_scr[:, :64], ep[:], ACTF.Square,
                             accum_out=ssq2[:, 0:1])

        # -- h_pre = hidden + 0.3 * (gv @ W_vel^T), fp32 --
        gvT = work.tile([64, P], F32, tag="gvT")
        ptg = psm.tile([P, D], F32, tag="ps")
        nc.tensor.matmul(ptg[:64, :P], gv[:], ident[:], is_transpose=True)
        nc.scalar.copy(gvT[:], ptg[:64, :P])
        pvel = psm.tile([P, D], F32, tag="ps")
        nc.tensor.matmul(pvel[:], gvT[:], w_velT[:], start=True, stop=True)
        hpre = work.tile([P, D], F32, tag="hpre")
        nc.vector.scalar_tensor_tensor(hpre[:], pvel[:], 0.3, hid[:],
                                       op0=ALU.mult, op1=ALU.add)
        nc.scalar.activation(sq_scr[:], hpre[:], ACTF.Square,
                             accum_out=ssq2[:, 1:2])

        rsq2 = _rsqrt(nc, nc.vector, nwt, ssq2, 2)
        esc = small.tile([P, 1], F32, tag="esc")
        nc.vector.tensor_scalar(esc[:], ent[:], 5.0 / LOGV, None, op0=ALU.mult)
        nc.vector.tensor_tensor(esc[:], esc[:], rsq2[:, 0:1], op=ALU.mult)
        nc.vector.tensor_scalar_mul(ep[:], ep[:], esc[:])
        hsc = small.tile([P, 1], F32, tag="hsc")
        nc.vector.tensor_scalar(hsc[:], rsq2[:, 1:2], 2.5, None, op0=ALU.mult)
        nc.vector.tensor_scalar_mul(hpre[:], hpre[:], hsc[:])

        # -- transposes: epT (f32r), haT (fp32), scn_qT raw + x5 --
        epT = work.tile([64, P], F32R, tag="epT")
        pte = psm.tile([P, D], F32, tag="ps")
        nc.tensor.matmul(pte[:64, :P], ep[:], ident[:], is_transpose=True)
        nc.vector.tensor_copy(epT[:], pte[:64, :P].bitcast(F32R))
        haT = work.tile([P, 2 * P], F32, tag="haT")
        for j in range(2):
            pt = psm.tile([P, D], F32, tag="ps")
            nc.tensor.matmul(pt[:, :P], hpre[:, j * P : (j + 1) * P], ident[:],
                             is_transpose=True)
            nc.scalar.copy(haT[:, j * P : (j + 1) * P], pt[:, :P])

        # ---- z_g PSUM: es(f32r) + hm(fp32) + mask(bf16) ----
        zg = pz.tile([P, S], F32, tag="z")
        for h in range(2):
            hs = slice(h * 512, (h + 1) * 512)
            nc.tensor.matmul(zg[:, hs], epT[:], scnT_r[:, hs],
                             start=True, stop=False, skip_group_check=True)
        for j in range(2):
            for h in range(2):
                hs = slice(h * 512, (h + 1) * 512)
                nc.tensor.matmul(zg[:, hs], haT[:, j * P : (j + 1) * P],
                                 msgsT[j][:, hs], start=False, stop=False,
                                 skip_group_check=True)
        for h in range(2):
            hs = slice(h * 512, (h + 1) * 512)
            nc.tensor.matmul(zg[:, hs], ident_bf[:], mask_bf[:, hs],
                             start=False, stop=True, skip_group_check=True)

        # ---- tau via top-8, both branches batched [P, 2, 8] ----
        t8 = small.tile([P, 16], F32, tag="t8")
        nc.vector.max(t8[:, 0:8], zg[:])
        nc.vector.max(t8[:, 8:16], zl[:])
        c8 = small.tile([P, 16], F32, tag="c8")
        d8 = small.tile([P, 16], F32, tag="d8")
        t8v = t8[:].rearrange("p (b k) -> p b k", b=2)
        c8v = c8[:].rearrange("p (b k) -> p b k", b=2)
        d8v = d8[:].rearrange("p (b k) -> p b k", b=2)
        nc.vector.tensor_copy(c8v[:, :, 0:1], t8v[:, :, 0:1])
        nc.vector.tensor_tensor(c8v[:, :, 1:8], t8v[:, :, 1:8], t8v[:, :, 0:7],
                                op=ALU.add)
        nc.vector.tensor_copy(d8v[:, :, 0:2], c8v[:, :, 0:2])
        nc.vector.tensor_tensor(d8v[:, :, 2:8], c8v[:, :, 2:8], c8v[:, :, 0:6],
                                op=ALU.add)
        nc.vector.tensor_copy(c8v[:, :, 0:4], d8v[:, :, 0:4])
        nc.vector.tensor_tensor(c8v[:, :, 4:8], d8v[:, :, 4:8], d8v[:, :, 0:4],
                                op=ALU.add)
        nc.vector.tensor_scalar(c8[:], c8[:], -1.0, None, op0=ALU.add)
        nc.vector.tensor_tensor(c8v[:], c8v[:],
                                rk8[:, :].unsqueeze(1).broadcast_to([P, 2, 8]),
                                op=ALU.mult)
        tau2 = small.tile([P, 2], F32, tag="tau2")
        nc.vector.tensor_reduce(tau2[:], c8v[:], axis=AXX, op=ALU.max)
        tau_g = tau2[:, 0:1]
        tau_l = tau2[:, 1:2]
        if STAGE < 2:
            outt = work.tile([P, D], F32, tag="outt")
            nc.vector.tensor_scalar_mul(outt[:], hid[:], tau_g[:])
            nc.vector.tensor_scalar_mul(outt[:], outt[:], tau_l[:])
            nc.sync.dma_start(out_dram[qsl, :], outt[:])
            continue

        # ---- W1' = relu(gw*zg - gw*tau_g), W2' = relu(bw*zl - bw*tau_l) ----
        nbias_g = small.tile([P, 1], F32, tag="nbg")
        nc.vector.tensor_tensor(nbias_g[:], gw[:], tau_g, op=ALU.mult)
        nc.vector.tensor_scalar(nbias_g[:], nbias_g[:], -1.0, None, op0=ALU.mult)
        nbias_l = small.tile([P, 1], F32, tag="nbl")
        nc.vector.tensor_tensor(nbias_l[:], bw[:], tau_l, op=ALU.mult)
        nc.vector.tensor_scalar(nbias_l[:], nbias_l[:], -1.0, None, op0=ALU.mult)
        w1 = work.tile([P, S], F32R, tag="w1")
        nc.scalar.activation(w1[:], zg[:], ACTF.Relu, bias=nbias_g[:], scale=gw[:])
        w2 = work.tile([P, S], F32R, tag="w2")
        nc.scalar.activation(w2[:], zl[:], ACTF.Relu, bias=nbias_l[:], scale=bw[:])

        if STAGE < 3:
            outt = work.tile([P, D], F32, tag="outt")
            nc.scalar.activation(outt[:], w1[:].bitcast(F32)[:, 0:D], ACTF.Copy)
            nc.vector.tensor_tensor(outt[:], outt[:], w2[:].bitcast(F32)[:, 0:D], op=ALU.add)
            nc.sync.dma_start(out_dram[qsl, :], outt[:])
            continue
        # ---- static branch ----
        g1 = work.tile([P, C], F32, tag="g1")
        nc.gpsimd.indirect_copy(g1[:], scnT[:, :], pk_w[:], True)
        g2 = work.tile([P, C], F32, tag="g2")
        nc.gpsimd.indirect_copy(g2[:], sqT[:, :], pq_w[:], True)
        pp = work.tile([C, C], F32, tag="pp")
        nc.vector.tensor_tensor(pp[:64, :], g1[:64, :], g2[:64, :], op=ALU.mult)
        ptsc = psm.tile([P, D], F32, tag="ps")
        nc.tensor.matmul(ptsc[:C, 0:1], pp[:64, :], ones64[:],
                         start=True, stop=True)
        simc = small.tile([C, 1], F32, tag="simc")
        nc.scalar.copy(simc[:], ptsc[:C, 0:1])
        krhs = small.tile([C, 8], F32, tag="krhs")
        nc.vector.tensor_scalar_mul(krhs[:], k_oh[:], simc[:])
        pss = psm.tile([P, D], F32, tag="ps")
        nc.tensor.matmul(pss[:, 0:8], s_oh[:], krhs[:], start=True, stop=True)
        sim_s = small.tile([P, 8], F32, tag="sims")
        nc.scalar.copy(sim_s[:], pss[:, 0:8])

        # edge MLP (batched): x2 = 0.5*(sim*W_e1[h] + b_e1[h]), h-major [P,64]
        x2 = work.tile([P, 64], F32, tag="x2")
        x2v = x2[:].rearrange("p (h k) -> p h k", h=8)
        sim_bc = sim_s[:].unsqueeze(1).broadcast_to([P, 8, 8])
        w1v = w_e1[:, :].rearrange("p (h k) -> p h k", h=8)
        b1v = b_e1[:, :].rearrange("p (h k) -> p h k", h=8)
        nc.vector.tensor_tensor(x2v, sim_bc, w1v, op=ALU.mult)
        nc.vector.tensor_tensor(x2v, x2v, b1v, op=ALU.add)
        # tanh-gelu: gelu(x) ~= x2*(1+tanh(c*(2*x2)*(1+0.17886*x2^2))), x2=x/2
        xsq = work.tile([P, 64], F32, tag="xsq")
        nc.vector.tensor_tensor(xsq[:], x2[:], x2[:], op=ALU.mult)
        nc.vector.tensor_scalar(xsq[:], xsq[:], 0.35772, 2.0, op0=ALU.mult,
                                op1=ALU.add)
        nc.vector.tensor_tensor(xsq[:], xsq[:], x2[:], op=ALU.mult)
        erf = work.tile([P, 64], F32, tag="erf")
        nc.scalar.activation(erf[:], xsq[:], ACTF.Tanh, scale=0.7978845608028654)
        he = work.tile([P, 64], F32, tag="he")
        nc.vector.scalar_tensor_tensor(he[:], erf[:], 1.0, x2[:],
                                       op0=ALU.add, op1=ALU.mult)
        nc.vector.tensor_tensor(he[:], he[:], w_e2[:, :], op=ALU.mult)
        edge = small.tile([P, 8], F32, tag="edge")
        nc.vector.tensor_reduce(
            edge[:], he[:].rearrange("p (h k) -> p k h", h=8),
            axis=AXX, op=ALU.add)
        nc.vector.tensor_scalar(edge[:], edge[:], b_e2[:], None, op0=ALU.add)
        # softmax over k via sigmoid-exp identity
        mx = small.tile([P, 1], F32, tag="mx")
        nc.vector.tensor_reduce(mx[:], edge[:], axis=AXX, op=ALU.max)
        sg = small.tile([P, 8], F32, tag="sg")
        nc.scalar.activation(sg[:], edge[:], ACTF.Sigmoid, bias=mx[:], scale=-1.0)
        ex = small.tile([P, 8], F32, tag="ex")
        nc.vector.reciprocal(ex[:], sg[:])
        nc.vector.tensor_scalar(ex[:], ex[:], -1.0, None, op0=ALU.add)
        den = small.tile([P, 1], F32, tag="den")
        nc.vector.tensor_reduce(den[:], ex[:], axis=AXX, op=ALU.add)
        rden = small.tile([P, 1], F32, tag="rden")
        nc.vector.reciprocal(rden[:], den[:])
        ews = small.tile([P, 8], F32, tag="ews")
        nc.vector.tensor_scalar_mul(ews[:], ex[:], rden[:])
        # scatter weights: w4d layout slot-major [si*8 + k]
        w4d = small.tile([P, 16], BF16, tag="w4d")
        ewsaw = small.tile([P, 8], F32, tag="ewsaw")
        nc.vector.tensor_scalar_mul(ewsaw[:], ews[:], aw[:])
        nc.vector.tensor_copy(w4d[:, 0:8], ewsaw[:])
        nc.vector.tensor_copy(w4d[:, 8:16], ewsaw[:])
        w4 = work.tile([P, C], BF16, tag="w4")
        nc.gpsimd.local_scatter(w4[:], w4d[:], w4i[:], channels=P,
                                num_elems=C, num_idxs=16)
        ptw = psm.tile([P, D], BF16, tag="ps")
        nc.tensor.matmul(ptw[:C, :P], w4[:], ident_bf[:], is_transpose=True)
        w4T = small.tile([C, P], BF16, tag="w4T")
        nc.vector.tensor_copy(w4T[:], ptw[:C, :P])
        ohp = work.tile([C, S], BF16, tag="ohp")
        nc.vector.tensor_scalar(ohp[:], iota16[:], pkcol[:], None, op0=ALU.is_equal)

        if STAGE < 4:
            outt = work.tile([P, D], F32, tag="outt")
            nc.vector.tensor_scalar_mul(outt[:], hid[:], aw[:])
            nc.vector.tensor_tensor(outt[:, 0:8], outt[:, 0:8], ews[:], op=ALU.add)
            nc.vector.tensor_tensor(outt[:, 0:64], outt[:, 0:64], w4[:].bitcast(F32)[:, 0:32].bitcast(BF16), op=ALU.add)
            nc.sync.dma_start(out_dram[qsl, :], outt[:])
            continue
        # ---- WcT accumulation: W1'^T + W2'^T + static oh-matmul ----
        wcT_ps = pz.tile([P, S], F32, tag="z")
        for i in range(8):
            sl = slice(i * P, (i + 1) * P)
            nc.tensor.matmul(wcT_ps[:, sl].bitcast(F32R), w1[:, sl], ident_r[:],
                             is_transpose=True, start=True, stop=False,
                             skip_group_check=True)
            nc.tensor.matmul(wcT_ps[:, sl].bitcast(F32R), w2[:, sl], ident_r[:],
                             is_transpose=True, start=False, stop=False,
                             skip_group_check=True)
            nc.tensor.matmul(wcT_ps[:, sl], ohp[:, sl], w4T[:],
                             start=False, stop=True, skip_group_check=True)
        wcT = work.tile([P, S], F32R, tag="wcTs")
        nc.scalar.copy(wcT[:], wcT_ps[:].bitcast(F32R))

        if STAGE < 5:
            outt = work.tile([P, D], F32, tag="outt")
            nc.scalar.activation(outt[:], wcT[:].bitcast(F32)[:, 0:D], ACTF.Copy)
            nc.sync.dma_start(out_dram[qsl, :], outt[:])
            continue
        # ---- agg (f32r) and probe (f32r) ----
        agg = psm.tile([P, D], F32, tag="ps")
        for i in range(8):
            nc.tensor.matmul(agg[:], wcT[:, i * P : (i + 1) * P],
                             msgs_r[:, i * D : (i + 1) * D],
                             start=(i == 0), stop=(i == 7))
        probe = psm.tile([P, D], F32, tag="ps")
        for j in range(2):
            nc.tensor.matmul(probe[:], hidT[:, j * P : (j + 1) * P],
                             w_probeT[j][:], start=(j == 0), stop=(j == 1))

        # ---- rel = sigmoid(dot * rsqrt(na2*np2)); out = agg * rel ----
        probe_sb = work.tile([P, D], F32, tag="prsb")
        nc.scalar.copy(probe_sb[:], probe[:])
        na2 = small.tile([P, 1], F32, tag="na2")
        sq_scr2 = work.tile([P, D], F32, tag="sqscr2")
        nc.scalar.activation(sq_scr2[:], agg[:], ACTF.Square, accum_out=na2[:])
        np2 = small.tile([P, 1], F32, tag="np2")
        sq_scr3 = work.tile([P, D], F32, tag="sqscr3")
        nc.scalar.activation(sq_scr3[:], probe_sb[:], ACTF.Square, accum_out=np2[:])
        dot = small.tile([P, 1], F32, tag="dot")
        dscr = work.tile([P, D], F32, tag="dscr")
        nc.vector.scalar_tensor_tensor(dscr[:], agg[:], 1.0, probe_sb[:],
                                       op0=ALU.mult, op1=ALU.mult,
                                       accum_out=dot[:])
        nn2 = small.tile([P, 1], F32, tag="nn2")
        nc.vector.tensor_tensor(nn2[:], na2[:], np2[:], op=ALU.mult)
        rsqn = _rsqrt(nc, nc.vector, nwt, nn2, 1)
        rel = small.tile([P, 1], F32, tag="rel")
        nc.vector.tensor_tensor(rel[:], dot[:], rsqn[:], op=ALU.mult)
        nc.scalar.activation(rel[:], rel[:], ACTF.Sigmoid)
        outt = work.tile([P, D], F32, tag="outt")
        nc.vector.tensor_scalar_mul(outt[:], agg[:], rel[:])
        nc.sync.dma_start(out_dram[qsl, :], outt[:])


# ------------------------------------------------------------------ host ----

_CACHE = {}


def _build_module(repeat: int = 1):
    key = f"nc{repeat}"
    if key in _CACHE:
        return _CACHE[key]
    nc = bacc.Bacc("TRN2", target_bir_lowering=False, debug=False, num_devices=8)

    def dt_(name, shape, dtype, kind="ExternalInput"):
        return nc.dram_tensor(name, shape, dtype, kind=kind).ap()

    io = {
        "messages": dt_("messages", [S, D], F32),
        "scn": dt_("scn", [S, 64], F32),
        "scn_q": dt_("scn_q", [Q, 64], F32),
        "hidden_q": dt_("hidden_q", [Q, D], F32),
        "gv_q": dt_("gv_q", [Q, 64], F32),
        "maskneg_q": dt_("maskneg_q", [Q, S], BF16),
        "ent_q": dt_("ent_q", [Q, 1], F32),
        "conf_q": dt_("conf_q", [Q, 1], F32),
        "W_vel": dt_("W_vel", [D, NS], F32),
        "W_probe": dt_("W_probe", [D, D], F32),
        "W_gate": dt_("W_gate", [1, D], F32),
        "W_e1c": dt_("W_e1c", [1, 64], F32),
        "b_e1c": dt_("b_e1c", [1, 64], F32),
        "W_e2c": dt_("W_e2c", [1, 64], F32),
        "b_e2c": dt_("b_e2c", [1, 1], F32),
        "b_gatec": dt_("b_gatec", [1, 1], F32),
        "rk8c": dt_("rk8c", [1, 8], F32),
        "iota16c": dt_("iota16c", [1, S], F32),
        "mcolc": dt_("mcolc", [C, 1], F32),
        "mcol65c": dt_("mcol65c", [65, 1], F32),
        "ohqscalec": dt_("ohqscalec", [65, 1], F32),
        "pk_col": dt_("pk_col", [NQT, C, 1], F32),
        "pk_wrap": dt_("pk_wrap", [NQT, 128, C // 16], U16),
        "pq_wrap": dt_("pq_wrap", [NQT, 128, C // 16], U16),
        "w4_idx": dt_("w4_idx", [NQT, 128, 16], I16),
        "s_oh": dt_("s_oh", [NQT, C, 128], F32),
        "k_oh": dt_("k_oh", [NQT, C, 8], F32),
        "coverage": dt_("coverage", [NQT, 128, 1], F32),
        "out": dt_("out", [Q, D], F32, kind="ExternalOutput"),
    }
    with tile.TileContext(nc) as tc:
        build_kernel(tc, io, repeat)
    nc.compile()
    _CACHE[key] = nc
    return nc


def _host_prep(inputs):
    import ml_dtypes
    msgs = np.ascontiguousarray(inputs["messages"], dtype=np.float32)
    hid = np.ascontiguousarray(inputs["hidden"], dtype=np.float32)
    x_ids = np.asarray(inputs["x_ids"]).astype(np.int64)
    scn = np.ascontiguousarray(inputs["scn"], dtype=np.float32)
    mask = np.asarray(inputs["mask"]).astype(bool)
    static_nb = np.asarray(inputs["static_nb"]).astype(np.int64)
    gv = np.ascontiguousarray(inputs["geo_velocity"], dtype=np.float32)
    conf = np.asarray(inputs["ctx_conf"]).astype(np.float32).reshape(B, S, 1)
    ent = np.asarray(inputs["current_entropy"]).astype(np.float32)

    blocked = mask | np.eye(S, dtype=bool)
    maskneg = (MASKNEG * blocked.astype(np.float32)).astype(ml_dtypes.bfloat16)

    rk8 = (1.0 / np.arange(1, 9, dtype=np.float32)).reshape(1, 8)
    iota16 = np.arange(S, dtype=np.float32).reshape(1, S)
    mcol = np.arange(C, dtype=np.float32).reshape(C, 1)
    mcol65 = np.full((65, 1), -1.0, np.float32)
    mcol65[:64, 0] = np.arange(64)
    ohqsc = np.full((65, 1), BIGNEG, np.float32)
    ohqsc[64, 0] = -BIGNEG

    shared = {
        "W_vel": np.ascontiguousarray(inputs["W_vel"], dtype=np.float32),
        "W_probe": np.ascontiguousarray(inputs["W_probe"], dtype=np.float32),
        "W_gate": np.ascontiguousarray(inputs["W_gate"], dtype=np.float32).reshape(1, D),
        "W_e1c": np.repeat(0.5 * np.asarray(inputs["W_e1"], np.float32).reshape(8), 8).reshape(1, 64),
        "b_e1c": np.repeat(0.5 * np.asarray(inputs["b_e1"], np.float32).reshape(8), 8).reshape(1, 64),
        "W_e2c": np.repeat(np.asarray(inputs["W_e2"], np.float32).reshape(8), 8).reshape(1, 64),
        "b_e2c": np.asarray(inputs["b_e2"], np.float32).reshape(1, 1),
        "b_gatec": np.asarray(inputs["b_gate"], np.float32).reshape(1, 1),
        "rk8c": rk8, "iota16c": iota16, "mcolc": mcol, "mcol65c": mcol65,
        "ohqscalec": ohqsc,
    }

    in_maps = []
    for core in range(8):
        b, half = divmod(core, 2)
        base = half * Q
        pos = {}
        for t, v in enumerate(x_ids[b]):
            pos.setdefault(int(v), []).append(t)
        nbv = static_nb[x_ids[b, base : base + Q]]

        pk_col = np.zeros((NQT, C, 1), np.float32)
        pk_wrap = np.zeros((NQT, 128, C // 16), np.uint16)
        pq_wrap = np.zeros((NQT, 128, C // 16), np.uint16)
        w4_idx = np.full((NQT, 128, 16), -1, np.int16)
        s_oh = np.zeros((NQT, C, 128), np.float32)
        k_oh = np.zeros((NQT, C, 8), np.float32)
        coverage = np.zeros((NQT, 128, 1), np.float32)

        for qt in range(NQT):
            pairs = []
            cnt = np.zeros((128, 8), np.float32)
            for s_loc in range(128):
                s_glob = base + qt * 128 + s_loc
                for k in range(8):
                    v = int(nbv[qt * 128 + s_loc, k])
                    ms = [t for t in pos.get(v, []) if t <= s_glob]
                    cnt[s_loc, k] = len(ms)
                    assert len(ms) <= 2, f"cnt>2: core{core} qt{qt}"
                    for si, t in enumerate(ms):
                        pairs.append((s_loc, k, t, si))
            assert len(pairs) <= C, f"{len(pairs)} pairs > C: core{core} qt{qt}"
            flatk = np.zeros(C, np.uint16)
            flatq = np.zeros(C, np.uint16)
            for j, (s_loc, k, t, si) in enumerate(pairs):
                pk_col[qt, j, 0] = t
                s_oh[qt, j, s_loc] = 1.0
                k_oh[qt, j, k] = 1.0
                w4_idx[qt, s_loc, si * 8 + k] = j
                flatk[j] = t
                flatq[j] = s_loc
            for p in range(128):
                for sw in range(C // 16):
                    pk_wrap[qt, p, sw] = flatk[sw * 16 + (p % 16)]
                    pq_wrap[qt, p, sw] = flatq[sw * 16 + (p % 16)]
            coverage[qt, :, 0] = np.minimum(cnt, 1.0).mean(-1)
            # unmatched pairs -> pk_col must not accidentally one-hot-match:
            for j in range(len(pairs), C):
                pk_col[qt, j, 0] = -1

        in_maps.append({
            **shared,
            "messages": msgs[b],
            "scn": scn[b],
            "scn_q": scn[b, base : base + Q],
            "hidden_q": hid[b, base : base + Q],
            "gv_q": gv[b, base : base + Q],
            "maskneg_q": np.ascontiguousarray(maskneg[base : base + Q]),
            "ent_q": ent[b, base : base + Q].reshape(Q, 1),
            "conf_q": conf[b, base : base + Q].reshape(Q, 1),
            "pk_col": pk_col, "pk_wrap": pk_wrap, "pq_wrap": pq_wrap,
            "w4_idx": w4_idx, "s_oh": s_oh, "k_oh": k_oh, "coverage": coverage,
        })
    return in_maps


def run(inputs, trace=False, repeat=1):
    in_maps = _host_prep(inputs)
    nc = _build_module(repeat)
    br = run_bass_kernel_spmd(nc, in_maps, list(range(8)), trace=trace)
    out = np.zeros((B, S, D), np.float32)
    for core in range(8):
        b, half = divmod(core, 2)
        out[b, half * Q : (half + 1) * Q] = br.results[core]["out"]
    return out, br


def kernel(**inputs):
    out, _ = run(inputs)
    return out
